# revision 12
# baseline (speedup 1.0000x reference)
"""GAT 2-layer kernel for 8 Trainium2 NeuronCores (v2).

Strategy (dst-sharded edge partitioning, engine-balanced):
  - Nodes and their in-edges sharded by dst across 8 cores (12500 nodes each).
    Self-loops appended as regular edges; edges sorted by dst window (98
    windows of 128 dsts), grouped into 4 src-banks (int16-indexable 32768-row
    overlapping bank views of the node table), chunk-padded to K=5 chunks of
    128 edges per (window, bank).
  - Per-node table rows [(1|xh_h*32) x4 | a_src(4) | pad] (512B stride) are
    computed on-device with dense matmuls (8 tiles per DMA, batched stores via
    a 3D DRAM access pattern); per-edge rows fetched with the gpsimd
    dma_gather custom op (int16 indices, bank-relative, 4 SWDGE queues).
  - e = a_src[src] + a_dst[dst] accumulated in PSUM per chunk by two matmuls:
    ohT.T @ a_dst_window (ohT = PE-transposed one-hot) + I.T @ rows[132:136].
    leaky-relu as one fused scalar_tensor_tensor; exp on the scalar engine.
  - Softmax without max-subtraction, normalization after aggregation:
    out[d] = (sum ex*xh) / (sum ex).  The interleaved (1|xh) row layout makes
    rhs = rows * ex_broadcast a single fused vector op; the leading-ones
    columns accumulate sum(ex) in the same scatter matmul.
  - Scatter within a window is a one-hot matmul into PSUM over 20 chunks.
  - Biases applied after normalization (phase 3); between layers an AllGather
    of fp16 transposed h shards feeds layer 2's table build.
"""

import numpy as np

# ---------------------------------------------------------------- constants
N = 100000
E_IN = 1600000
CORES = 8
M = N // CORES              # 12500 nodes per core
P = 128
WPC = (M + P - 1) // P      # 98 windows per core
H, C = 4, 32                # heads x channels (both layers)
F = 128                     # feature width (= H*C)
ROW = 256                   # f16 elements per table row (512B)
RW = 136                    # used columns: 4*(1+32) + 4 a_src
BANKS = 4
BANK_ROWS = 32768
BANK_BASES = [0, 22411, 44822, 67232]
PAGE = 8                    # chunks per dma_gather call (1024 idx)
NEG_SLOPE = 0.2
EPS = 1e-30

NPAD = 100352               # N padded to 8*12544 (= 98 groups of 1024)
MPAD = 12544                # per-core padded node count (98*128)
XSPAD = 13312               # xTs padded to 13*1024


def _host_prep(edge_index):
    """Partition/sort/bank/pad the edge structure. Returns per-core data:
      idxw   [128, BANKS*NCALLB*S] int16 -- wrapped bank-relative gather idx
      dstrel [128, NCHUNKS]        f32   -- dst lane relative to window (-1 pad)
    """
    src = np.concatenate([edge_index[0], np.arange(N, dtype=np.int64)]).astype(np.int64)
    dst = np.concatenate([edge_index[1], np.arange(N, dtype=np.int64)]).astype(np.int64)

    bases = np.asarray(BANK_BASES, np.int64)
    cores = []
    maxK = 0
    for m in range(CORES):
        sel = (dst // M) == m
        s_m = src[sel]
        dloc = (dst[sel] - m * M).astype(np.int64)
        win = dloc // P
        order = np.argsort(win, kind="stable")
        s_m, dloc, win = s_m[order], dloc[order], win[order]
        hi_b = np.searchsorted(bases, s_m, side="right") - 1
        lo_ok = (hi_b > 0) & (s_m < bases[np.maximum(hi_b - 1, 0)] + BANK_ROWS)
        lo_b = np.where(lo_ok, hi_b - 1, hi_b)
        bank = np.empty(len(s_m), np.int8)
        wstarts = np.searchsorted(win, np.arange(WPC + 1))
        for w in range(WPC):
            a, z = wstarts[w], wstarts[w + 1]
            nb = z - a
            T = -(-nb // BANKS)
            cnt = np.bincount(hi_b[a:z][~lo_ok[a:z]], minlength=BANKS).astype(np.int64)
            bw = hi_b[a:z].copy()
            rigid = ~lo_ok[a:z]
            bw[rigid] = hi_b[a:z][rigid]
            for pnr in range(BANKS - 1):
                fm = lo_ok[a:z] & (lo_b[a:z] == pnr)
                f = int(fm.sum())
                give = min(f, max(0, T - int(cnt[pnr])))
                idxs = np.flatnonzero(fm)
                bw[idxs[:give]] = pnr
                bw[idxs[give:]] = pnr + 1
                cnt[pnr] += give
                cnt[pnr + 1] += f - give
            bank[a:z] = bw
            maxK = max(maxK, int(-(-cnt.max() // P)))
        cores.append((s_m, dloc, win, bank, wstarts))

    K = max(5, maxK)
    CPB = WPC * K
    NCALLB = (CPB + PAGE - 1) // PAGE
    NCHUNKS = WPC * BANKS * K

    out = []
    for m in range(CORES):
        s_m, dloc, win, bank, wstarts = cores[m]
        idx_flat = np.zeros((BANKS, CPB * P), np.int16)
        rel_flat = np.full((BANKS, CPB * P), -1.0, np.float32)
        for w in range(WPC):
            a, z = wstarts[w], wstarts[w + 1]
            bw = bank[a:z]
            for b in range(BANKS):
                mask = bw == b
                sl = (s_m[a:z][mask] - BANK_BASES[b]).astype(np.int16)
                rl = (dloc[a:z][mask] - w * P).astype(np.float32)
                base = (w * K) * P
                idx_flat[b, base:base + len(sl)] = sl
                rel_flat[b, base:base + len(rl)] = rl
        NIDX = PAGE * P
        S = NIDX // 16
        idxw = np.zeros((P, BANKS * NCALLB * S), np.int16)
        for b in range(BANKS):
            for j in range(NCALLB):
                seg = np.zeros(NIDX, np.int16)
                have = idx_flat[b, j * NIDX:(j + 1) * NIDX]
                seg[:len(have)] = have
                w16 = seg.reshape(S, 16).T
                col0 = (b * NCALLB + j) * S
                idxw[:, col0:col0 + S] = np.tile(w16, (CORES, 1))
        dstrel = np.full((P, NCHUNKS), -1.0, np.float32)
        for w in range(WPC):
            for b in range(BANKS):
                for k in range(K):
                    cid = (w * BANKS + b) * K + k
                    seg = rel_flat[b, (w * K + k) * P:(w * K + k + 1) * P]
                    dstrel[:, cid] = seg
        out.append({"idxw": idxw, "dstrel": dstrel})
    consts = {"K": K, "CPB": CPB, "NCALLB": NCALLB, "NCHUNKS": NCHUNKS,
              "NIDX": PAGE * P, "S": (PAGE * P) // 16}
    return out, consts


def _pack_mats(W, att_src, att_dst):
    """Weight matrix packed for the interleaved row layout.
    Returns wcat [F_in, 136] (cols h*33 zero, h*33+1+c = W col h*32+c,
    132+h = W@att_src_h) and wad [F_in, 4] (= W@att_dst)."""
    F_in = W.shape[0]
    W = W.astype(np.float32)
    wcat = np.zeros((F_in, RW), np.float32)
    for h in range(H):
        wcat[:, h * 33 + 1:(h + 1) * 33] = W[:, h * C:(h + 1) * C]
        wcat[:, 132 + h] = W[:, h * C:(h + 1) * C] @ att_src[h].astype(np.float32)
    wad = np.zeros((F_in, H), np.float32)
    for h in range(H):
        wad[:, h] = W[:, h * C:(h + 1) * C] @ att_dst[h].astype(np.float32)
    return wcat, wad


def emulate(inputs, perm, consts):
    """Numpy emulation of the device algorithm (layout-faithful, f32 math)."""
    K = consts["K"]
    x = np.asarray(inputs["x"], np.float32)
    b1 = np.asarray(inputs["b1"], np.float32)
    b2 = np.asarray(inputs["b2"], np.float32)
    wcat1, wad1 = _pack_mats(np.asarray(inputs["W1"], np.float32),
                             np.asarray(inputs["att_src1"]), np.asarray(inputs["att_dst1"]))
    wcat2, wad2 = _pack_mats(np.asarray(inputs["W2"], np.float32),
                             np.asarray(inputs["att_src2"]), np.asarray(inputs["att_dst2"]))
    brow = np.zeros(RW, np.float32)
    for h in range(H):
        brow[h * 33] = 1.0

    def layer(xin, wcat, wad, bias, concat):
        t = (xin @ wcat + brow).astype(np.float16)   # [N, 136] table
        ad = (xin @ wad).astype(np.float16)          # [N, 4]
        outs = []
        for m in range(CORES):
            pw = np.zeros((MPAD, 132), np.float64)
            idxw, dstrel = perm[m]["idxw"], perm[m]["dstrel"]
            S, NCALLB, CPB = consts["S"], consts["NCALLB"], consts["CPB"]
            for b in range(BANKS):
                for j in range(NCALLB):
                    col0 = (b * NCALLB + j) * S
                    seg = idxw[:16, col0:col0 + S].T.reshape(-1)
                    for pg in range(PAGE):
                        cglob = j * PAGE + pg
                        if cglob >= CPB:
                            break
                        w, k = cglob // K, cglob % K
                        cid = (w * BANKS + b) * K + k
                        lanes = seg[pg * P:(pg + 1) * P].astype(np.int64) + BANK_BASES[b]
                        rows = t[lanes].astype(np.float32)
                        rel = dstrel[:, cid]
                        valid = rel >= 0
                        d = np.where(valid, rel, 0).astype(np.int64) + w * P
                        e = rows[:, 132:136] + ad[m * M + np.minimum(d, M - 1)].astype(np.float32)
                        e = np.where(e >= 0, e, NEG_SLOPE * e)
                        ex = np.exp(e).astype(np.float16).astype(np.float32)
                        rhs = (rows[:, 0:132].reshape(P, H, 33)
                               * ex[:, :, None]).astype(np.float16).astype(np.float32)
                        np.add.at(pw, d, rhs.reshape(P, 132) * valid[:, None])
            pw = pw[:M]
            s = pw.reshape(M, H, 33)[:, :, 0] + EPS
            unn = pw.reshape(M, H, 33)[:, :, 1:33]
            o = unn / s[:, :, None]
            o = o.reshape(M, F) + (bias if concat else 0)
            outs.append(o.astype(np.float32))
        full = np.concatenate(outs, axis=0)
        return full

    h1 = layer(x, wcat1, wad1, b1, True)
    h1 = np.where(h1 > 0, h1, np.expm1(h1)).astype(np.float16).astype(np.float32)
    o2 = layer(h1, wcat2, wad2, None, False)
    o2 = o2.reshape(N, H, C).mean(axis=1) + b2
    return o2.astype(np.float32)


# ======================================================================
# device program (Bass/Tile)
# ======================================================================
import concourse.bacc as bacc
import concourse.bass as bass
import concourse.mybir as mybir
import concourse.tile as tile
from concourse.tile import ScopedClock
from concourse.masks import make_identity
from concourse.bass_utils import run_bass_kernel_spmd

F16 = mybir.dt.float16
F32 = mybir.dt.float32
I16 = mybir.dt.int16
AF = mybir.ActivationFunctionType
ALU = mybir.AluOpType
NGRP = NPAD // 1024          # 98 phase-0 groups of 8 tiles
NT8 = 8                      # tiles per group

# ---------------------------------------------------------------- drain patch
# walrus allows at most ONE sync wait on CTRL/DMA instructions, but the Tile
# kernel-tail drain waits on every DMA sem lane used (up to 16). Split them.
def _patched_drain_and_barrier(self, tick_clock, wait_clock):
    drain_inst = self.nc.sync.drain()
    wait_clock.add_sem_waits(
        drain_inst.ins, ScopedClock({None: tick_clock.global_clock})
    )
    si = drain_inst.ins.sync_info
    waits = list(si.on_wait or []) if si is not None else []
    if len(waits) > 1:
        si.on_wait = waits[:1]
        for w in waits[1:]:
            extra = self.nc.sync.drain()
            esi = extra.ins.sync_info
            if esi is None:
                import bass_rust
                extra.ins.sync_info = bass_rust.SyncInfo(on_wait=[], on_update=[])
                esi = extra.ins.sync_info
            esi.on_wait = [w]
    self.nc.all_engine_barrier()
    assert self.sems is not None
    popped = self.nc._tile_sem_poison_stack.pop()
    assert popped is self._sem_poison
    self.nc.clear_and_free_semaphores(list(self.sems.allocated().values()))
    self.nc.all_engine_barrier()

tile.TileContext._drain_and_barrier = _patched_drain_and_barrier


_NC_CACHE = {}


def build(consts):
    ck = tuple(sorted(consts.items()))
    if ck in _NC_CACHE:
        return _NC_CACHE[ck]
    K = consts["K"]
    CPB = consts["CPB"]
    NCALLB = consts["NCALLB"]
    NCHUNKS = consts["NCHUNKS"]
    NIDX = consts["NIDX"]
    S = consts["S"]
    CW = BANKS * K               # chunks per window (20)

    nc = bacc.Bacc("TRN2", target_bir_lowering=False, debug=False,
                   num_devices=CORES, num_swdge_queues=4)

    # ------------------------------------------------------------- tensors
    xT = nc.dram_tensor("xT", [P, NPAD], F16, kind="ExternalInput")
    xTs = nc.dram_tensor("xTs", [P, XSPAD], F16, kind="ExternalInput")
    wcat1 = nc.dram_tensor("wcat1", [P, RW], F16, kind="ExternalInput")
    wcat2 = nc.dram_tensor("wcat2", [P, RW], F16, kind="ExternalInput")
    wad1 = nc.dram_tensor("wad1", [P, H], F16, kind="ExternalInput")
    wad2 = nc.dram_tensor("wad2", [P, H], F16, kind="ExternalInput")
    brow = nc.dram_tensor("brow", [1, RW], F16, kind="ExternalInput")
    b1t = nc.dram_tensor("b1t", [P, F], F32, kind="ExternalInput")
    b2t = nc.dram_tensor("b2t", [P, C], F32, kind="ExternalInput")
    idxw = nc.dram_tensor("idxw", [P, BANKS * NCALLB * S], I16, kind="ExternalInput")
    dstrel = nc.dram_tensor("dstrel", [P, NCHUNKS], F32, kind="ExternalInput")
    out2 = nc.dram_tensor("out2", [MPAD, C], F32, kind="ExternalOutput")

    table = [nc.dram_tensor(f"table{l}", [NPAD, ROW], F16) for l in (1, 2)]
    h_shard = nc.dram_tensor("h_shard", [P, MPAD], F16)
    h_full = nc.dram_tensor("h_full", [CORES, P, MPAD], F16, addr_space="Shared")

    with tile.TileContext(nc) as tc:
        with (
            tc.tile_pool(name="const", bufs=1) as cpool,
            tc.tile_pool(name="resident", bufs=1) as rpool,
            tc.tile_pool(name="p0", bufs=4) as p0pool,
            tc.tile_pool(name="p0st", bufs=3) as p0st,
            tc.tile_pool(name="p0ps", bufs=2, space="PSUM") as p0ps,
            tc.tile_pool(name="gat", bufs=3) as gpool,
            tc.tile_pool(name="oh", bufs=2 * CW + 4) as ohpool,
            tc.tile_pool(name="cmp", bufs=3) as cmppool,
            tc.tile_pool(name="wps", bufs=2, space="PSUM") as wps,
            tc.tile_pool(name="eps", bufs=2, space="PSUM") as epsp,
            tc.tile_pool(name="trps", bufs=2, space="PSUM") as trps,
            tc.tile_pool(name="p3", bufs=3) as p3pool,
            tc.tile_pool(name="stg", bufs=2) as stgpool,
        ):
            # ---------------- constants
            ident = cpool.tile([P, P], F16)
            make_identity(nc, ident[:])
            iota_i = cpool.tile([P, P], mybir.dt.int32)
            nc.gpsimd.iota(iota_i[:], pattern=[[1, P]], base=0, channel_multiplier=0)
            iota_row = cpool.tile([P, P], F16)
            nc.vector.tensor_copy(iota_row[:], iota_i[:])
            ones_row = cpool.tile([1, P], F16)
            nc.vector.memset(ones_row[:], 1.0)

            wc = []
            for l, t in ((0, wcat1), (1, wcat2)):
                w_t = cpool.tile([P, RW], F16, tag=f"wc{l}")
                nc.sync.dma_start(out=w_t[:], in_=t[:, :])
                wc.append(w_t)
            wad = []
            for l, t in ((0, wad1), (1, wad2)):
                w_t = cpool.tile([P, H], F16, tag=f"wad{l}")
                nc.sync.dma_start(out=w_t[:], in_=t[:, :])
                wad.append(w_t)
            brow_t = cpool.tile([1, RW], F16)
            nc.sync.dma_start(out=brow_t[:], in_=brow[:, :])
            b1_t = cpool.tile([P, F], F32)
            nc.sync.dma_start(out=b1_t[:], in_=b1t[:, :])
            b2_t = cpool.tile([P, C], F32)
            nc.sync.dma_start(out=b2_t[:], in_=b2t[:, :])

            idx_t = rpool.tile([P, BANKS * NCALLB * S], I16)
            nc.sync.dma_start(out=idx_t[:], in_=idxw[:, :])
            rel_t = rpool.tile([P, NCHUNKS], F32)
            nc.sync.dma_start(out=rel_t[:], in_=dstrel[:, :])

            adres0 = rpool.tile([P, WPC * H], F16, tag="ad0")
            adres1 = rpool.tile([P, WPC * H], F16, tag="ad1")
            adres = [adres0, adres1]

            # layer-1 a_dst from the local xT shard (batched loads)
            for g in range(13):
                wlo = g * 8
                nwin = min(8, WPC - wlo)
                if nwin <= 0:
                    break
                lt = p0pool.tile([P, 1024], F16, tag="adl")
                nc.sync.dma_start(out=lt[:], in_=xTs[:, g * 1024:(g + 1) * 1024])
                aps = epsp.tile([P, CW * H], F32, tag="ep")
                for k in range(nwin):
                    nc.tensor.matmul(aps[:, k * H:(k + 1) * H],
                                     lhsT=lt[:, k * P:(k + 1) * P], rhs=wad[0][:],
                                     start=True, stop=True)
                nc.vector.tensor_copy(adres0[:, wlo * H:(wlo + nwin) * H],
                                      aps[:, 0:nwin * H])

            for L in range(2):
                # ======================================================= phase 0
                for g in range(NGRP):
                    lt = p0pool.tile([P, 1024], F16, tag="p0l")
                    if L == 0:
                        nc.sync.dma_start(out=lt[:], in_=xT[:, g * 1024:(g + 1) * 1024])
                    else:
                        # table rows are REAL-node indexed; h_full blocks hold
                        # 12500 real cols (+44 pad) each — split at 12500s.
                        n0 = g * 1024
                        done = 0
                        while done < 1024:
                            nr = n0 + done
                            if nr >= N:
                                nc.vector.memset(lt[:, done:1024], 0.0)
                                break
                            blk = nr // M
                            off = nr % M
                            take = min(1024 - done, M - off, N - nr)
                            nc.sync.dma_start(
                                out=lt[:, done:done + take],
                                in_=h_full[blk, :, off:off + take])
                            done += take
                    stage = p0st.tile([P, NT8, RW], F16, tag="st")
                    # 8 matmul pairs into 3-wide PSUM tiles (bank limit 2KB);
                    # batched PSUM->SBUF f16 copies on the scalar engine
                    for k0 in (0, 3, 6):
                        kn = min(3, NT8 - k0)
                        ps = p0ps.tile([P, 3, RW], F32, tag="p0p")
                        for kk in range(kn):
                            k = k0 + kk
                            nc.tensor.matmul(ps[:, kk, :],
                                             lhsT=lt[:, k * P:(k + 1) * P],
                                             rhs=wc[L][:], start=True, stop=False)
                            nc.tensor.matmul(ps[:, kk, :], lhsT=ones_row[:1, :],
                                             rhs=brow_t[:], start=False, stop=True)
                        nc.scalar.activation(stage[:, k0:k0 + kn, :],
                                             ps[:, 0:kn, :], AF.Copy)
                    nc.sync.dma_start(
                        out=table[L][g * 1024:(g + 1) * 1024, 0:RW].rearrange(
                            "(k p) c -> p k c", p=P),
                        in_=stage[:])

                # ======================================================= edges
                # Software-pipelined by one window: phase 3 of window w-1 is
                # emitted between pass A and pass B of window w, so the vector
                # engine never stalls on the PE/Act round trips of phase 3.
                nextcall = [0] * BANKS
                gtiles = [dict() for _ in range(BANKS)]
                stg = {"h": None, "o": None}

                def pass_a(w):
                    for b in range(BANKS):
                        while nextcall[b] * PAGE < min((w + 1) * K, CPB):
                            j = nextcall[b]
                            gt = gpool.tile([P, PAGE, ROW], F16, tag=f"g{b}")
                            col0 = (b * NCALLB + j) * S
                            nc.gpsimd.dma_gather(
                                gt[:], table[L][BANK_BASES[b]:BANK_BASES[b] + BANK_ROWS, :],
                                idx_t[:, col0:col0 + S], NIDX, NIDX, ROW,
                                queue_num=b)
                            gtiles[b][j] = gt
                            if j - 2 in gtiles[b]:
                                del gtiles[b][j - 2]
                            nextcall[b] += 1
                    eps = epsp.tile([P, CW * H], F32, tag="ep")
                    ohs = []
                    for cw in range(CW):
                        cid = (w * BANKS + (cw // K)) * K + (cw % K)
                        oh = ohpool.tile([P, P], F16, tag="oh")
                        nc.vector.tensor_scalar(
                            oh[:], iota_row[:], rel_t[:, cid:cid + 1], None,
                            op0=ALU.is_equal)
                        ohs.append(oh)
                    ohTs = []
                    for b in range(BANKS):
                        trp = trps.tile([P, K * P], F16, tag="tr")
                        for k in range(K):
                            nc.tensor.transpose(trp[:, k * P:(k + 1) * P],
                                                ohs[b * K + k][:], ident[:])
                        ohT = cmppool.tile([P, K * P], F16, tag="ohT")
                        nc.scalar.activation(ohT[:], trp[:], AF.Copy)
                        ohTs.append(ohT)
                    # paired open/close per chunk: PSUM tracks only one open
                    # accumulation group per bank, so each chunk's two matmuls
                    # (ohT.T@adres then I.T@rows_as) must be adjacent.
                    for cw in range(CW):
                        b, k = cw // K, cw % K
                        cglob = w * K + k
                        gt = gtiles[b][cglob // PAGE]
                        nc.tensor.matmul(eps[:, cw * H:(cw + 1) * H],
                                         lhsT=ohTs[b][:, k * P:(k + 1) * P],
                                         rhs=adres[L][:, w * H:(w + 1) * H],
                                         start=True, stop=False)
                        nc.tensor.matmul(eps[:, cw * H:(cw + 1) * H],
                                         lhsT=ident[:],
                                         rhs=gt[:, cglob % PAGE, 132:136],
                                         start=False, stop=True)
                    return eps, ohs

                def pass_b(w, eps, ohs):
                    # leaky-relu (only one PSUM operand allowed per DVE op)
                    lrs = p3pool.tile([P, CW * H], F32, tag="lrs")
                    nc.vector.tensor_scalar(lrs[:], eps[:], NEG_SLOPE, None,
                                            op0=ALU.mult)
                    lr = p3pool.tile([P, CW * H], F32, tag="lr")
                    nc.vector.tensor_tensor(lr[:], lrs[:], eps[:], op=ALU.max)
                    ex = p3pool.tile([P, CW * H], F16, tag="ex")
                    nc.scalar.activation(ex[:], lr[:], AF.Exp)
                    pw = wps.tile([P, RW], F32, tag="pw")
                    for cw in range(CW):
                        b, k = cw // K, cw % K
                        cglob = w * K + k
                        rows = gtiles[b][cglob // PAGE][:, cglob % PAGE, :]
                        rhs = cmppool.tile([P, 132], F16, tag="rhs")
                        nc.vector.tensor_tensor(
                            rhs[:].rearrange("p (h c) -> p h c", h=H),
                            rows[:, 0:132].rearrange("p (h c) -> p h c", h=H),
                            ex[:, cw * H:(cw + 1) * H, None].to_broadcast([P, H, 33]),
                            op=ALU.mult)
                        nc.tensor.matmul(pw[:, 0:132], lhsT=ohs[cw][:], rhs=rhs[:],
                                         start=(cw == 0), stop=(cw == CW - 1))
                    return pw

                def phase3(w, pw):
                    sp = p3pool.tile([P, H], F32, tag="s")
                    nc.scalar.activation(
                        sp[:, :, None],
                        pw[:, 0:132].rearrange("p (h c) -> p h c", h=H)[:, :, 0:1],
                        AF.Copy, bias=EPS)
                    r = p3pool.tile([P, H], F32, tag="r")
                    nc.vector.reciprocal(r[:], sp[:])
                    hw = p3pool.tile([P, F], F32, tag="hw")
                    nc.vector.tensor_tensor(
                        hw[:].rearrange("p (h c) -> p h c", h=H),
                        pw[:, 0:132].rearrange("p (h c) -> p h c", h=H)[:, :, 1:33],
                        r[:, :, None].to_broadcast([P, H, C]),
                        op=ALU.mult)
                    if L == 0:
                        if w % 8 == 0:
                            hs_new = stgpool.tile([P, 1024], F16, tag="hs")
                            stg["h"] = hs_new
                        hstage = stg["h"]
                        nc.vector.tensor_tensor(hw[:], hw[:], b1_t[:], op=ALU.add)
                        # elu
                        mn = p3pool.tile([P, F], F32, tag="mn")
                        nc.vector.tensor_scalar(mn[:], hw[:], 0.0, None, op0=ALU.min)
                        mx = p3pool.tile([P, F], F32, tag="mx")
                        nc.vector.tensor_scalar(mx[:], hw[:], 0.0, None, op0=ALU.max)
                        ek = p3pool.tile([P, F], F32, tag="ek")
                        nc.scalar.activation(ek[:], mn[:], AF.Exp)
                        he = p3pool.tile([P, F], F16, tag="he")
                        nc.vector.scalar_tensor_tensor(
                            he[:], ek[:], -1.0, mx[:], op0=ALU.add, op1=ALU.add)
                        trp = trps.tile([P, K * P], F16, tag="tr")
                        nc.tensor.transpose(trp[:, 0:P], he[:], ident[:])
                        hcol = (w % 8) * P
                        nc.scalar.activation(hstage[:, hcol:hcol + P], trp[:, 0:P],
                                             AF.Copy)
                        adp = epsp.tile([P, CW * H], F32, tag="ep")
                        nc.tensor.matmul(adp[:, 0:H],
                                         lhsT=hstage[:, hcol:hcol + P], rhs=wad[1][:],
                                         start=True, stop=True)
                        nc.vector.tensor_copy(adres1[:, w * H:(w + 1) * H], adp[:, 0:H])
                        if w % 8 == 7 or w == WPC - 1:
                            g0 = (w // 8) * 1024
                            gw = min(1024, MPAD - g0)
                            nc.sync.dma_start(out=h_shard[:, g0:g0 + gw],
                                              in_=hstage[:, 0:gw])
                    else:
                        if w % 8 == 0:
                            os_new = stgpool.tile([P, NT8, C], F32, tag="os")
                            stg["o"] = os_new
                        ostage = stg["o"]
                        red = p3pool.tile([P, C], F32, tag="red")
                        nc.vector.tensor_reduce(
                            red[:], hw[:].rearrange("p (h c) -> p c h", h=H),
                            axis=mybir.AxisListType.X, op=ALU.add)
                        nc.vector.scalar_tensor_tensor(
                            ostage[:, w % 8, :], red[:], 1.0 / H, b2_t[:],
                            op0=ALU.mult, op1=ALU.add)
                        if w % 8 == 7 or w == WPC - 1:
                            g0 = (w // 8) * 1024
                            kw = (w % 8) + 1
                            nc.sync.dma_start(
                                out=out2[g0:g0 + kw * P, :].rearrange(
                                    "(k p) c -> p k c", p=P),
                                in_=ostage[:, 0:kw, :])

                prev = None
                for w in range(WPC):
                    eps, ohs = pass_a(w)
                    if prev is not None:
                        phase3(w - 1, prev)
                    prev = pass_b(w, eps, ohs)
                phase3(WPC - 1, prev)

                if L == 0:
                    nc.gpsimd.collective_compute(
                        "AllGather", ALU.bypass,
                        replica_groups=[list(range(CORES))],
                        ins=[h_shard.ap()],
                        outs=[h_full.ap()],
                    )
    nc.compile()
    _NC_CACHE[ck] = nc
    return nc


def make_inmaps(inputs, perm, consts):
    x = np.asarray(inputs["x"], np.float32)
    wcat1, wad1 = _pack_mats(np.asarray(inputs["W1"], np.float32),
                             np.asarray(inputs["att_src1"]), np.asarray(inputs["att_dst1"]))
    wcat2, wad2 = _pack_mats(np.asarray(inputs["W2"], np.float32),
                             np.asarray(inputs["att_src2"]), np.asarray(inputs["att_dst2"]))
    b1 = np.asarray(inputs["b1"], np.float32)
    b2 = np.asarray(inputs["b2"], np.float32)
    brow_np = np.zeros((1, RW), np.float16)
    for h in range(H):
        brow_np[0, h * 33] = 1.0
    xT_np = np.zeros((P, NPAD), np.float16)
    xT_np[:, :N] = np.ascontiguousarray(x.T).astype(np.float16)
    common = {
        "xT": xT_np,
        "wcat1": wcat1.astype(np.float16), "wcat2": wcat2.astype(np.float16),
        "wad1": wad1.astype(np.float16), "wad2": wad2.astype(np.float16),
        "brow": brow_np,
        "b1t": np.tile(b1[None, :], (P, 1)).astype(np.float32),
        "b2t": np.tile(b2[None, :], (P, 1)).astype(np.float32),
    }
    maps = []
    for m in range(CORES):
        im = dict(common)
        xs = np.zeros((P, XSPAD), np.float16)
        xs[:, :M] = xT_np[:, m * M:(m + 1) * M]
        im["xTs"] = xs
        im["idxw"] = perm[m]["idxw"]
        im["dstrel"] = perm[m]["dstrel"]
        maps.append(im)
    return maps


def run_on_hw(inputs, perm, consts):
    nc = build(consts)
    maps = make_inmaps(inputs, perm, consts)
    res = run_bass_kernel_spmd(nc, maps, core_ids=list(range(CORES)))
    return np.concatenate([res.results[m]["out2"][:M] for m in range(CORES)], axis=0)


def kernel(**inputs):
    perm, consts = _host_prep(np.asarray(inputs["edge_index"]))
    out = run_on_hw(inputs, perm, consts)
    if not np.isfinite(out).all():
        # transient first-dispatch flakiness: retry once
        out = run_on_hw(inputs, perm, consts)
    return out


# revision 20
# speedup vs baseline: 1.0045x; 1.0045x over previous
"""GAT 2-layer kernel for 8 Trainium2 NeuronCores (v2).

Strategy (dst-sharded edge partitioning, engine-balanced):
  - Nodes and their in-edges sharded by dst across 8 cores (12500 nodes each).
    Self-loops appended as regular edges; edges sorted by dst window (98
    windows of 128 dsts), grouped into 4 src-banks (int16-indexable 32768-row
    overlapping bank views of the node table), chunk-padded to K=5 chunks of
    128 edges per (window, bank).
  - Per-node table rows [(1|xh_h*32) x4 | a_src(4) | pad] (512B stride) are
    computed on-device with dense matmuls (8 tiles per DMA, batched stores via
    a 3D DRAM access pattern); per-edge rows fetched with the gpsimd
    dma_gather custom op (int16 indices, bank-relative, 4 SWDGE queues).
  - e = a_src[src] + a_dst[dst] accumulated in PSUM per chunk by two matmuls:
    ohT.T @ a_dst_window (ohT = PE-transposed one-hot) + I.T @ rows[132:136].
    leaky-relu as one fused scalar_tensor_tensor; exp on the scalar engine.
  - Softmax without max-subtraction, normalization after aggregation:
    out[d] = (sum ex*xh) / (sum ex).  The interleaved (1|xh) row layout makes
    rhs = rows * ex_broadcast a single fused vector op; the leading-ones
    columns accumulate sum(ex) in the same scatter matmul.
  - Scatter within a window is a one-hot matmul into PSUM over 20 chunks.
  - Biases applied after normalization (phase 3); between layers an AllGather
    of fp16 transposed h shards feeds layer 2's table build.
"""

import numpy as np

# ---------------------------------------------------------------- constants
N = 100000
E_IN = 1600000
CORES = 8
M = N // CORES              # 12500 nodes per core
P = 128
WPC = (M + P - 1) // P      # 98 windows per core
H, C = 4, 32                # heads x channels (both layers)
F = 128                     # feature width (= H*C)
ROW = 256                   # f16 elements per table row (512B)
RW = 136                    # used columns: 4*(1+32) + 4 a_src
BANKS = 4
BANK_ROWS = 32768
BANK_BASES = [0, 22411, 44822, 67232]
PAGE = 8                    # chunks per dma_gather call (1024 idx)
NEG_SLOPE = 0.2
EPS = 1e-30

NPAD = 100352               # N padded to 8*12544 (= 98 groups of 1024)
MPAD = 12544                # per-core padded node count (98*128)
XSPAD = 13312               # xTs padded to 13*1024


def _host_prep(edge_index):
    """Partition/sort/bank/pad the edge structure. Returns per-core data:
      idxw   [128, BANKS*NCALLB*S] int16 -- wrapped bank-relative gather idx
      dstrel [128, NCHUNKS]        f32   -- dst lane relative to window (-1 pad)
    """
    src = np.concatenate([edge_index[0], np.arange(N, dtype=np.int64)]).astype(np.int64)
    dst = np.concatenate([edge_index[1], np.arange(N, dtype=np.int64)]).astype(np.int64)

    bases = np.asarray(BANK_BASES, np.int64)
    cores = []
    maxK = 0
    for m in range(CORES):
        sel = (dst // M) == m
        s_m = src[sel]
        dloc = (dst[sel] - m * M).astype(np.int64)
        win = dloc // P
        order = np.argsort(win, kind="stable")
        s_m, dloc, win = s_m[order], dloc[order], win[order]
        hi_b = np.searchsorted(bases, s_m, side="right") - 1
        lo_ok = (hi_b > 0) & (s_m < bases[np.maximum(hi_b - 1, 0)] + BANK_ROWS)
        lo_b = np.where(lo_ok, hi_b - 1, hi_b)
        bank = np.empty(len(s_m), np.int8)
        wstarts = np.searchsorted(win, np.arange(WPC + 1))
        for w in range(WPC):
            a, z = wstarts[w], wstarts[w + 1]
            nb = z - a
            T = -(-nb // BANKS)
            cnt = np.bincount(hi_b[a:z][~lo_ok[a:z]], minlength=BANKS).astype(np.int64)
            bw = hi_b[a:z].copy()
            rigid = ~lo_ok[a:z]
            bw[rigid] = hi_b[a:z][rigid]
            for pnr in range(BANKS - 1):
                fm = lo_ok[a:z] & (lo_b[a:z] == pnr)
                f = int(fm.sum())
                give = min(f, max(0, T - int(cnt[pnr])))
                idxs = np.flatnonzero(fm)
                bw[idxs[:give]] = pnr
                bw[idxs[give:]] = pnr + 1
                cnt[pnr] += give
                cnt[pnr + 1] += f - give
            bank[a:z] = bw
            maxK = max(maxK, int(-(-cnt.max() // P)))
        cores.append((s_m, dloc, win, bank, wstarts))

    K = max(5, maxK)
    CPB = WPC * K
    NCALLB = (CPB + PAGE - 1) // PAGE
    NCHUNKS = WPC * BANKS * K

    out = []
    for m in range(CORES):
        s_m, dloc, win, bank, wstarts = cores[m]
        idx_flat = np.zeros((BANKS, CPB * P), np.int16)
        rel_flat = np.full((BANKS, CPB * P), -1.0, np.float32)
        for w in range(WPC):
            a, z = wstarts[w], wstarts[w + 1]
            bw = bank[a:z]
            for b in range(BANKS):
                mask = bw == b
                sl = (s_m[a:z][mask] - BANK_BASES[b]).astype(np.int16)
                rl = (dloc[a:z][mask] - w * P).astype(np.float32)
                base = (w * K) * P
                idx_flat[b, base:base + len(sl)] = sl
                rel_flat[b, base:base + len(rl)] = rl
        NIDX = PAGE * P
        S = NIDX // 16
        idxw = np.zeros((P, BANKS * NCALLB * S), np.int16)
        for b in range(BANKS):
            for j in range(NCALLB):
                seg = np.zeros(NIDX, np.int16)
                have = idx_flat[b, j * NIDX:(j + 1) * NIDX]
                seg[:len(have)] = have
                w16 = seg.reshape(S, 16).T
                col0 = (b * NCALLB + j) * S
                idxw[:, col0:col0 + S] = np.tile(w16, (CORES, 1))
        dstrel = np.full((P, NCHUNKS), -1.0, np.float32)
        for w in range(WPC):
            for b in range(BANKS):
                for k in range(K):
                    cid = (w * BANKS + b) * K + k
                    seg = rel_flat[b, (w * K + k) * P:(w * K + k + 1) * P]
                    dstrel[:, cid] = seg
        out.append({"idxw": idxw, "dstrel": dstrel})
    consts = {"K": K, "CPB": CPB, "NCALLB": NCALLB, "NCHUNKS": NCHUNKS,
              "NIDX": PAGE * P, "S": (PAGE * P) // 16}
    return out, consts


def _pack_mats(W, att_src, att_dst):
    """Weight matrix packed for the interleaved row layout.
    Returns wcat [F_in, 136] (cols h*33 zero, h*33+1+c = W col h*32+c,
    132+h = W@att_src_h) and wad [F_in, 4] (= W@att_dst)."""
    F_in = W.shape[0]
    W = W.astype(np.float32)
    wcat = np.zeros((F_in, RW), np.float32)
    for h in range(H):
        wcat[:, h * 33 + 1:(h + 1) * 33] = W[:, h * C:(h + 1) * C]
        wcat[:, 132 + h] = W[:, h * C:(h + 1) * C] @ att_src[h].astype(np.float32)
    wad = np.zeros((F_in, H), np.float32)
    for h in range(H):
        wad[:, h] = W[:, h * C:(h + 1) * C] @ att_dst[h].astype(np.float32)
    return wcat, wad


def emulate(inputs, perm, consts):
    """Numpy emulation of the device algorithm (layout-faithful, f32 math)."""
    K = consts["K"]
    x = np.asarray(inputs["x"], np.float32)
    b1 = np.asarray(inputs["b1"], np.float32)
    b2 = np.asarray(inputs["b2"], np.float32)
    wcat1, wad1 = _pack_mats(np.asarray(inputs["W1"], np.float32),
                             np.asarray(inputs["att_src1"]), np.asarray(inputs["att_dst1"]))
    wcat2, wad2 = _pack_mats(np.asarray(inputs["W2"], np.float32),
                             np.asarray(inputs["att_src2"]), np.asarray(inputs["att_dst2"]))
    brow = np.zeros(RW, np.float32)
    for h in range(H):
        brow[h * 33] = 1.0

    def layer(xin, wcat, wad, bias, concat):
        t = (xin @ wcat + brow).astype(np.float16)   # [N, 136] table
        ad = (xin @ wad).astype(np.float16)          # [N, 4]
        outs = []
        for m in range(CORES):
            pw = np.zeros((MPAD, 132), np.float64)
            idxw, dstrel = perm[m]["idxw"], perm[m]["dstrel"]
            S, NCALLB, CPB = consts["S"], consts["NCALLB"], consts["CPB"]
            for b in range(BANKS):
                for j in range(NCALLB):
                    col0 = (b * NCALLB + j) * S
                    seg = idxw[:16, col0:col0 + S].T.reshape(-1)
                    for pg in range(PAGE):
                        cglob = j * PAGE + pg
                        if cglob >= CPB:
                            break
                        w, k = cglob // K, cglob % K
                        cid = (w * BANKS + b) * K + k
                        lanes = seg[pg * P:(pg + 1) * P].astype(np.int64) + BANK_BASES[b]
                        rows = t[lanes].astype(np.float32)
                        rel = dstrel[:, cid]
                        valid = rel >= 0
                        d = np.where(valid, rel, 0).astype(np.int64) + w * P
                        e = rows[:, 132:136] + ad[m * M + np.minimum(d, M - 1)].astype(np.float32)
                        e = np.where(e >= 0, e, NEG_SLOPE * e)
                        ex = np.exp(e).astype(np.float16).astype(np.float32)
                        rhs = (rows[:, 0:132].reshape(P, H, 33)
                               * ex[:, :, None]).astype(np.float16).astype(np.float32)
                        np.add.at(pw, d, rhs.reshape(P, 132) * valid[:, None])
            pw = pw[:M]
            s = pw.reshape(M, H, 33)[:, :, 0] + EPS
            unn = pw.reshape(M, H, 33)[:, :, 1:33]
            o = unn / s[:, :, None]
            o = o.reshape(M, F) + (bias if concat else 0)
            outs.append(o.astype(np.float32))
        full = np.concatenate(outs, axis=0)
        return full

    h1 = layer(x, wcat1, wad1, b1, True)
    h1 = np.where(h1 > 0, h1, np.expm1(h1)).astype(np.float16).astype(np.float32)
    o2 = layer(h1, wcat2, wad2, None, False)
    o2 = o2.reshape(N, H, C).mean(axis=1) + b2
    return o2.astype(np.float32)


# ======================================================================
# device program (Bass/Tile)
# ======================================================================
import concourse.bacc as bacc
import concourse.bass as bass
import concourse.mybir as mybir
import concourse.tile as tile
from concourse.tile import ScopedClock
from concourse.masks import make_identity
from concourse.bass_utils import run_bass_kernel_spmd

F16 = mybir.dt.float16
F32 = mybir.dt.float32
I16 = mybir.dt.int16
AF = mybir.ActivationFunctionType
ALU = mybir.AluOpType
NGRP = NPAD // 1024          # 98 phase-0 groups of 8 tiles
NT8 = 8                      # tiles per group

# ---------------------------------------------------------------- drain patch
# walrus allows at most ONE sync wait on CTRL/DMA instructions, but the Tile
# kernel-tail drain waits on every DMA sem lane used (up to 16). Split them.
def _patched_drain_and_barrier(self, tick_clock, wait_clock):
    drain_inst = self.nc.sync.drain()
    wait_clock.add_sem_waits(
        drain_inst.ins, ScopedClock({None: tick_clock.global_clock})
    )
    si = drain_inst.ins.sync_info
    waits = list(si.on_wait or []) if si is not None else []
    if len(waits) > 1:
        si.on_wait = waits[:1]
        for w in waits[1:]:
            extra = self.nc.sync.drain()
            esi = extra.ins.sync_info
            if esi is None:
                import bass_rust
                extra.ins.sync_info = bass_rust.SyncInfo(on_wait=[], on_update=[])
                esi = extra.ins.sync_info
            esi.on_wait = [w]
    self.nc.all_engine_barrier()
    assert self.sems is not None
    popped = self.nc._tile_sem_poison_stack.pop()
    assert popped is self._sem_poison
    self.nc.clear_and_free_semaphores(list(self.sems.allocated().values()))
    self.nc.all_engine_barrier()

tile.TileContext._drain_and_barrier = _patched_drain_and_barrier


_NC_CACHE = {}


def build(consts):
    ck = tuple(sorted(consts.items()))
    if ck in _NC_CACHE:
        return _NC_CACHE[ck]
    K = consts["K"]
    CPB = consts["CPB"]
    NCALLB = consts["NCALLB"]
    NCHUNKS = consts["NCHUNKS"]
    NIDX = consts["NIDX"]
    S = consts["S"]
    CW = BANKS * K               # chunks per window (20)

    nc = bacc.Bacc("TRN2", target_bir_lowering=False, debug=False,
                   num_devices=CORES, num_swdge_queues=4)

    # ------------------------------------------------------------- tensors
    xT = nc.dram_tensor("xT", [P, NPAD], F16, kind="ExternalInput")
    xTs = nc.dram_tensor("xTs", [P, XSPAD], F16, kind="ExternalInput")
    wcat1 = nc.dram_tensor("wcat1", [P, RW], F16, kind="ExternalInput")
    wcat2 = nc.dram_tensor("wcat2", [P, RW], F16, kind="ExternalInput")
    wad1 = nc.dram_tensor("wad1", [P, H], F16, kind="ExternalInput")
    wad2 = nc.dram_tensor("wad2", [P, H], F16, kind="ExternalInput")
    brow = nc.dram_tensor("brow", [1, RW], F16, kind="ExternalInput")
    b1t = nc.dram_tensor("b1t", [P, F], F32, kind="ExternalInput")
    b2t = nc.dram_tensor("b2t", [P, C], F32, kind="ExternalInput")
    idxw = nc.dram_tensor("idxw", [P, BANKS * NCALLB * S], I16, kind="ExternalInput")
    dstrel = nc.dram_tensor("dstrel", [P, NCHUNKS], F32, kind="ExternalInput")
    out2 = nc.dram_tensor("out2", [MPAD, C], F32, kind="ExternalOutput")

    table = [nc.dram_tensor(f"table{l}", [NPAD, ROW], F16) for l in (1, 2)]
    h_shard = nc.dram_tensor("h_shard", [P, MPAD], F16)
    h_full = nc.dram_tensor("h_full", [CORES, P, MPAD], F16, addr_space="Shared")

    with tile.TileContext(nc) as tc:
        with (
            tc.tile_pool(name="const", bufs=1) as cpool,
            tc.tile_pool(name="resident", bufs=1) as rpool,
            tc.tile_pool(name="p0", bufs=4) as p0pool,
            tc.tile_pool(name="p0st", bufs=4) as p0st,
            tc.tile_pool(name="p0ps", bufs=2, space="PSUM") as p0ps,
            tc.tile_pool(name="gat", bufs=4) as gpool,
            tc.tile_pool(name="oh", bufs=2 * CW + 4) as ohpool,
            tc.tile_pool(name="cmp", bufs=5) as cmppool,
            tc.tile_pool(name="wps", bufs=2, space="PSUM") as wps,
            tc.tile_pool(name="eps", bufs=2, space="PSUM") as epsp,
            tc.tile_pool(name="trps", bufs=2, space="PSUM") as trps,
            tc.tile_pool(name="p3", bufs=4) as p3pool,
            tc.tile_pool(name="stg", bufs=2) as stgpool,
        ):
            # ---------------- constants
            ident = cpool.tile([P, P], F16)
            make_identity(nc, ident[:])
            iota_i = cpool.tile([P, P], mybir.dt.int32)
            nc.gpsimd.iota(iota_i[:], pattern=[[1, P]], base=0, channel_multiplier=0)
            iota_row = cpool.tile([P, P], F16)
            nc.vector.tensor_copy(iota_row[:], iota_i[:])
            ones_row = cpool.tile([1, P], F16)
            nc.vector.memset(ones_row[:], 1.0)

            wc = []
            for l, t in ((0, wcat1), (1, wcat2)):
                w_t = cpool.tile([P, RW], F16, tag=f"wc{l}")
                nc.sync.dma_start(out=w_t[:], in_=t[:, :])
                wc.append(w_t)
            wad = []
            for l, t in ((0, wad1), (1, wad2)):
                w_t = cpool.tile([P, H], F16, tag=f"wad{l}")
                nc.sync.dma_start(out=w_t[:], in_=t[:, :])
                wad.append(w_t)
            brow_t = cpool.tile([1, RW], F16)
            nc.sync.dma_start(out=brow_t[:], in_=brow[:, :])
            b1_t = cpool.tile([P, F], F32)
            nc.sync.dma_start(out=b1_t[:], in_=b1t[:, :])
            b2_t = cpool.tile([P, C], F32)
            nc.sync.dma_start(out=b2_t[:], in_=b2t[:, :])

            idx_t = rpool.tile([P, BANKS * NCALLB * S], I16)
            nc.sync.dma_start(out=idx_t[:], in_=idxw[:, :])
            rel_t = rpool.tile([P, NCHUNKS], F32)
            nc.sync.dma_start(out=rel_t[:], in_=dstrel[:, :])

            adres0 = rpool.tile([P, WPC * H], F16, tag="ad0")
            adres1 = rpool.tile([P, WPC * H], F16, tag="ad1")
            adres = [adres0, adres1]

            # layer-1 a_dst from the local xT shard (batched loads)
            for g in range(13):
                wlo = g * 8
                nwin = min(8, WPC - wlo)
                if nwin <= 0:
                    break
                lt = p0pool.tile([P, 1024], F16, tag="adl")
                nc.sync.dma_start(out=lt[:], in_=xTs[:, g * 1024:(g + 1) * 1024])
                aps = epsp.tile([P, CW * H], F32, tag="ep")
                for k in range(nwin):
                    nc.tensor.matmul(aps[:, k * H:(k + 1) * H],
                                     lhsT=lt[:, k * P:(k + 1) * P], rhs=wad[0][:],
                                     start=True, stop=True)
                nc.vector.tensor_copy(adres0[:, wlo * H:(wlo + nwin) * H],
                                      aps[:, 0:nwin * H])

            for L in range(2):
                # ======================================================= phase 0
                for g in range(NGRP):
                    lt = p0pool.tile([P, 1024], F16, tag="p0l")
                    if L == 0:
                        nc.sync.dma_start(out=lt[:], in_=xT[:, g * 1024:(g + 1) * 1024])
                    else:
                        # table rows are REAL-node indexed; h_full blocks hold
                        # 12500 real cols (+44 pad) each — split at 12500s.
                        n0 = g * 1024
                        done = 0
                        while done < 1024:
                            nr = n0 + done
                            if nr >= N:
                                nc.vector.memset(lt[:, done:1024], 0.0)
                                break
                            blk = nr // M
                            off = nr % M
                            take = min(1024 - done, M - off, N - nr)
                            nc.sync.dma_start(
                                out=lt[:, done:done + take],
                                in_=h_full[blk, :, off:off + take])
                            done += take
                    stage = p0st.tile([P, NT8, RW], F16, tag="st")
                    # 8 matmul pairs into 3-wide PSUM tiles (bank limit 2KB);
                    # batched PSUM->SBUF f16 copies on the scalar engine
                    for k0 in (0, 3, 6):
                        kn = min(3, NT8 - k0)
                        ps = p0ps.tile([P, 3, RW], F32, tag="p0p")
                        for kk in range(kn):
                            k = k0 + kk
                            nc.tensor.matmul(ps[:, kk, :],
                                             lhsT=lt[:, k * P:(k + 1) * P],
                                             rhs=wc[L][:], start=True, stop=False)
                            nc.tensor.matmul(ps[:, kk, :], lhsT=ones_row[:1, :],
                                             rhs=brow_t[:], start=False, stop=True)
                        nc.scalar.activation(stage[:, k0:k0 + kn, :],
                                             ps[:, 0:kn, :], AF.Copy)
                    nc.sync.dma_start(
                        out=table[L][g * 1024:(g + 1) * 1024, 0:RW].rearrange(
                            "(k p) c -> p k c", p=P),
                        in_=stage[:])

                # ======================================================= edges
                # Software-pipelined by one window: phase 3 of window w-1 is
                # emitted between pass A and pass B of window w, so the vector
                # engine never stalls on the PE/Act round trips of phase 3.
                nextcall = [0] * BANKS
                gtiles = [dict() for _ in range(BANKS)]
                stg = {"h": None, "o": None}

                def pass_a(w):
                    for b in range(BANKS):
                        while nextcall[b] * PAGE < min((w + 1) * K, CPB):
                            j = nextcall[b]
                            gt = gpool.tile([P, PAGE, ROW], F16, tag=f"g{b}")
                            col0 = (b * NCALLB + j) * S
                            nc.gpsimd.dma_gather(
                                gt[:], table[L][BANK_BASES[b]:BANK_BASES[b] + BANK_ROWS, :],
                                idx_t[:, col0:col0 + S], NIDX, NIDX, ROW,
                                queue_num=b)
                            gtiles[b][j] = gt
                            if j - 2 in gtiles[b]:
                                del gtiles[b][j - 2]
                            nextcall[b] += 1
                    eps = epsp.tile([P, CW * H], F32, tag="ep")
                    ohs = []
                    for cw in range(CW):
                        cid = (w * BANKS + (cw // K)) * K + (cw % K)
                        oh = ohpool.tile([P, P], F16, tag="oh")
                        nc.vector.tensor_scalar(
                            oh[:], iota_row[:], rel_t[:, cid:cid + 1], None,
                            op0=ALU.is_equal)
                        ohs.append(oh)
                    ohTs = []
                    for b in range(BANKS):
                        trp = trps.tile([P, K * P], F16, tag="tr")
                        for k in range(K):
                            nc.tensor.transpose(trp[:, k * P:(k + 1) * P],
                                                ohs[b * K + k][:], ident[:])
                        ohT = cmppool.tile([P, K * P], F16, tag="ohT")
                        nc.scalar.activation(ohT[:], trp[:], AF.Copy)
                        ohTs.append(ohT)
                    # paired open/close per chunk: PSUM tracks only one open
                    # accumulation group per bank, so each chunk's two matmuls
                    # (ohT.T@adres then I.T@rows_as) must be adjacent.
                    for cw in range(CW):
                        b, k = cw // K, cw % K
                        cglob = w * K + k
                        gt = gtiles[b][cglob // PAGE]
                        nc.tensor.matmul(eps[:, cw * H:(cw + 1) * H],
                                         lhsT=ohTs[b][:, k * P:(k + 1) * P],
                                         rhs=adres[L][:, w * H:(w + 1) * H],
                                         start=True, stop=False)
                        nc.tensor.matmul(eps[:, cw * H:(cw + 1) * H],
                                         lhsT=ident[:],
                                         rhs=gt[:, cglob % PAGE, 132:136],
                                         start=False, stop=True)
                    return eps, ohs

                def pass_b(w, eps, ohs):
                    # exp(lrelu(e)) = max(exp(e), exp(0.2 e)) since exp is
                    # monotonic: two scalar-engine exps + one f16 vector max
                    exa = p3pool.tile([P, CW * H], F16, tag="exa")
                    nc.scalar.activation(exa[:], eps[:], AF.Exp)
                    exb = p3pool.tile([P, CW * H], F16, tag="exb")
                    nc.scalar.activation(exb[:], eps[:], AF.Exp, scale=NEG_SLOPE)
                    ex = p3pool.tile([P, CW * H], F16, tag="ex")
                    nc.vector.tensor_tensor(ex[:], exa[:], exb[:], op=ALU.max)
                    pw = wps.tile([P, RW], F32, tag="pw")
                    for cw in range(CW):
                        b, k = cw // K, cw % K
                        cglob = w * K + k
                        rows = gtiles[b][cglob // PAGE][:, cglob % PAGE, :]
                        rhs = cmppool.tile([P, 132], F16, tag="rhs")
                        nc.vector.tensor_tensor(
                            rhs[:].rearrange("p (h c) -> p h c", h=H),
                            rows[:, 0:132].rearrange("p (h c) -> p h c", h=H),
                            ex[:, cw * H:(cw + 1) * H, None].to_broadcast([P, H, 33]),
                            op=ALU.mult)
                        nc.tensor.matmul(pw[:, 0:132], lhsT=ohs[cw][:], rhs=rhs[:],
                                         start=(cw == 0), stop=(cw == CW - 1))
                    return pw

                def phase3(w, pw):
                    sp = p3pool.tile([P, H], F32, tag="s")
                    nc.scalar.activation(
                        sp[:, :, None],
                        pw[:, 0:132].rearrange("p (h c) -> p h c", h=H)[:, :, 0:1],
                        AF.Copy, bias=EPS)
                    r = p3pool.tile([P, H], F32, tag="r")
                    nc.vector.reciprocal(r[:], sp[:])
                    hw = p3pool.tile([P, F], F32, tag="hw")
                    nc.vector.tensor_tensor(
                        hw[:].rearrange("p (h c) -> p h c", h=H),
                        pw[:, 0:132].rearrange("p (h c) -> p h c", h=H)[:, :, 1:33],
                        r[:, :, None].to_broadcast([P, H, C]),
                        op=ALU.mult)
                    if L == 0:
                        if w % 8 == 0:
                            hs_new = stgpool.tile([P, 1024], F16, tag="hs")
                            stg["h"] = hs_new
                        hstage = stg["h"]
                        nc.vector.tensor_tensor(hw[:], hw[:], b1_t[:], op=ALU.add)
                        # elu
                        mn = p3pool.tile([P, F], F32, tag="mn")
                        nc.vector.tensor_scalar(mn[:], hw[:], 0.0, None, op0=ALU.min)
                        mx = p3pool.tile([P, F], F32, tag="mx")
                        nc.vector.tensor_scalar(mx[:], hw[:], 0.0, None, op0=ALU.max)
                        ek = p3pool.tile([P, F], F32, tag="ek")
                        nc.scalar.activation(ek[:], mn[:], AF.Exp)
                        he = p3pool.tile([P, F], F16, tag="he")
                        nc.vector.scalar_tensor_tensor(
                            he[:], ek[:], -1.0, mx[:], op0=ALU.add, op1=ALU.add)
                        trp = trps.tile([P, K * P], F16, tag="tr")
                        nc.tensor.transpose(trp[:, 0:P], he[:], ident[:])
                        hcol = (w % 8) * P
                        nc.scalar.activation(hstage[:, hcol:hcol + P], trp[:, 0:P],
                                             AF.Copy)
                        adp = epsp.tile([P, CW * H], F32, tag="ep")
                        nc.tensor.matmul(adp[:, 0:H],
                                         lhsT=hstage[:, hcol:hcol + P], rhs=wad[1][:],
                                         start=True, stop=True)
                        nc.vector.tensor_copy(adres1[:, w * H:(w + 1) * H], adp[:, 0:H])
                        if w % 8 == 7 or w == WPC - 1:
                            g0 = (w // 8) * 1024
                            gw = min(1024, MPAD - g0)
                            nc.sync.dma_start(out=h_shard[:, g0:g0 + gw],
                                              in_=hstage[:, 0:gw])
                    else:
                        if w % 8 == 0:
                            os_new = stgpool.tile([P, NT8, C], F32, tag="os")
                            stg["o"] = os_new
                        ostage = stg["o"]
                        red = p3pool.tile([P, C], F32, tag="red")
                        nc.vector.tensor_reduce(
                            red[:], hw[:].rearrange("p (h c) -> p c h", h=H),
                            axis=mybir.AxisListType.X, op=ALU.add)
                        nc.vector.scalar_tensor_tensor(
                            ostage[:, w % 8, :], red[:], 1.0 / H, b2_t[:],
                            op0=ALU.mult, op1=ALU.add)
                        if w % 8 == 7 or w == WPC - 1:
                            g0 = (w // 8) * 1024
                            kw = (w % 8) + 1
                            nc.sync.dma_start(
                                out=out2[g0:g0 + kw * P, :].rearrange(
                                    "(k p) c -> p k c", p=P),
                                in_=ostage[:, 0:kw, :])

                prev = None
                for w in range(WPC):
                    eps, ohs = pass_a(w)
                    if prev is not None:
                        phase3(w - 1, prev)
                    prev = pass_b(w, eps, ohs)
                phase3(WPC - 1, prev)

                if L == 0:
                    nc.gpsimd.collective_compute(
                        "AllGather", ALU.bypass,
                        replica_groups=[list(range(CORES))],
                        ins=[h_shard.ap()],
                        outs=[h_full.ap()],
                    )
    nc.compile()
    _NC_CACHE[ck] = nc
    return nc


def make_inmaps(inputs, perm, consts):
    x = np.asarray(inputs["x"], np.float32)
    wcat1, wad1 = _pack_mats(np.asarray(inputs["W1"], np.float32),
                             np.asarray(inputs["att_src1"]), np.asarray(inputs["att_dst1"]))
    wcat2, wad2 = _pack_mats(np.asarray(inputs["W2"], np.float32),
                             np.asarray(inputs["att_src2"]), np.asarray(inputs["att_dst2"]))
    b1 = np.asarray(inputs["b1"], np.float32)
    b2 = np.asarray(inputs["b2"], np.float32)
    brow_np = np.zeros((1, RW), np.float16)
    for h in range(H):
        brow_np[0, h * 33] = 1.0
    xT_np = np.zeros((P, NPAD), np.float16)
    xT_np[:, :N] = np.ascontiguousarray(x.T).astype(np.float16)
    common = {
        "xT": xT_np,
        "wcat1": wcat1.astype(np.float16), "wcat2": wcat2.astype(np.float16),
        "wad1": wad1.astype(np.float16), "wad2": wad2.astype(np.float16),
        "brow": brow_np,
        "b1t": np.tile(b1[None, :], (P, 1)).astype(np.float32),
        "b2t": np.tile(b2[None, :], (P, 1)).astype(np.float32),
    }
    maps = []
    for m in range(CORES):
        im = dict(common)
        xs = np.zeros((P, XSPAD), np.float16)
        xs[:, :M] = xT_np[:, m * M:(m + 1) * M]
        im["xTs"] = xs
        im["idxw"] = perm[m]["idxw"]
        im["dstrel"] = perm[m]["dstrel"]
        maps.append(im)
    return maps


def run_on_hw(inputs, perm, consts):
    nc = build(consts)
    maps = make_inmaps(inputs, perm, consts)
    res = run_bass_kernel_spmd(nc, maps, core_ids=list(range(CORES)))
    return np.concatenate([res.results[m]["out2"][:M] for m in range(CORES)], axis=0)


def kernel(**inputs):
    perm, consts = _host_prep(np.asarray(inputs["edge_index"]))
    out = run_on_hw(inputs, perm, consts)
    if not np.isfinite(out).all():
        # transient first-dispatch flakiness: retry once
        out = run_on_hw(inputs, perm, consts)
    return out


# revision 24
# speedup vs baseline: 1.0979x; 1.0929x over previous
"""GAT 2-layer kernel for 8 Trainium2 NeuronCores (v2).

Strategy (dst-sharded edge partitioning, engine-balanced):
  - Nodes and their in-edges sharded by dst across 8 cores (12500 nodes each).
    Self-loops appended as regular edges; edges sorted by dst window (98
    windows of 128 dsts), grouped into 4 src-banks (int16-indexable 32768-row
    overlapping bank views of the node table), chunk-padded to K=5 chunks of
    128 edges per (window, bank).
  - Per-node table rows [(1|xh_h*32) x4 | a_src(4) | pad] (512B stride) are
    computed on-device with dense matmuls (8 tiles per DMA, batched stores via
    a 3D DRAM access pattern); per-edge rows fetched with the gpsimd
    dma_gather custom op (int16 indices, bank-relative, 4 SWDGE queues).
  - e = a_src[src] + a_dst[dst] accumulated in PSUM per chunk by two matmuls:
    ohT.T @ a_dst_window (ohT = PE-transposed one-hot) + I.T @ rows[132:136].
    leaky-relu as one fused scalar_tensor_tensor; exp on the scalar engine.
  - Softmax without max-subtraction, normalization after aggregation:
    out[d] = (sum ex*xh) / (sum ex).  The interleaved (1|xh) row layout makes
    rhs = rows * ex_broadcast a single fused vector op; the leading-ones
    columns accumulate sum(ex) in the same scatter matmul.
  - Scatter within a window is a one-hot matmul into PSUM over 20 chunks.
  - Biases applied after normalization (phase 3); between layers an AllGather
    of fp16 transposed h shards feeds layer 2's table build.
"""

import numpy as np

# ---------------------------------------------------------------- constants
N = 100000
E_IN = 1600000
CORES = 8
M = N // CORES              # 12500 nodes per core
P = 128
WPC = (M + P - 1) // P      # 98 windows per core
H, C = 4, 32                # heads x channels (both layers)
F = 128                     # feature width (= H*C)
ROW = 256                   # f16 elements per table row (512B)
RW = 136                    # used columns: 4*(1+32) + 4 a_src
BANKS = 4
BANK_ROWS = 32768
BANK_BASES = [0, 22411, 44822, 67232]
PAGE = 8                    # chunks per dma_gather call (1024 idx)
NEG_SLOPE = 0.2
EPS = 1e-30

NPAD = 100352               # N padded to 8*12544 (= 98 groups of 1024)
MPAD = 12544                # per-core padded node count (98*128)
XSPAD = 13312               # xTs padded to 13*1024


def _host_prep(edge_index):
    """Partition/sort/bank/pad the edge structure. Returns per-core data:
      idxw   [128, BANKS*NCALLB*S] int16 -- wrapped bank-relative gather idx
      dstrel [128, NCHUNKS]        f32   -- dst lane relative to window (-1 pad)
    """
    src = np.concatenate([edge_index[0], np.arange(N, dtype=np.int64)]).astype(np.int64)
    dst = np.concatenate([edge_index[1], np.arange(N, dtype=np.int64)]).astype(np.int64)

    bases = np.asarray(BANK_BASES, np.int64)
    cores = []
    maxK = 0
    for m in range(CORES):
        sel = (dst // M) == m
        s_m = src[sel]
        dloc = (dst[sel] - m * M).astype(np.int64)
        win = dloc // P
        order = np.argsort(win, kind="stable")
        s_m, dloc, win = s_m[order], dloc[order], win[order]
        hi_b = np.searchsorted(bases, s_m, side="right") - 1
        lo_ok = (hi_b > 0) & (s_m < bases[np.maximum(hi_b - 1, 0)] + BANK_ROWS)
        lo_b = np.where(lo_ok, hi_b - 1, hi_b)
        bank = np.empty(len(s_m), np.int8)
        wstarts = np.searchsorted(win, np.arange(WPC + 1))
        for w in range(WPC):
            a, z = wstarts[w], wstarts[w + 1]
            nb = z - a
            T = -(-nb // BANKS)
            cnt = np.bincount(hi_b[a:z][~lo_ok[a:z]], minlength=BANKS).astype(np.int64)
            bw = hi_b[a:z].copy()
            rigid = ~lo_ok[a:z]
            bw[rigid] = hi_b[a:z][rigid]
            for pnr in range(BANKS - 1):
                fm = lo_ok[a:z] & (lo_b[a:z] == pnr)
                f = int(fm.sum())
                give = min(f, max(0, T - int(cnt[pnr])))
                idxs = np.flatnonzero(fm)
                bw[idxs[:give]] = pnr
                bw[idxs[give:]] = pnr + 1
                cnt[pnr] += give
                cnt[pnr + 1] += f - give
            bank[a:z] = bw
            maxK = max(maxK, int(-(-cnt.max() // P)))
        cores.append((s_m, dloc, win, bank, wstarts))

    K = max(5, maxK)
    CPB = WPC * K
    NCALLB = (CPB + PAGE - 1) // PAGE
    NCHUNKS = WPC * BANKS * K

    out = []
    for m in range(CORES):
        s_m, dloc, win, bank, wstarts = cores[m]
        idx_flat = np.zeros((BANKS, CPB * P), np.int16)
        rel_flat = np.full((BANKS, CPB * P), -1.0, np.float32)
        for w in range(WPC):
            a, z = wstarts[w], wstarts[w + 1]
            bw = bank[a:z]
            for b in range(BANKS):
                mask = bw == b
                sl = (s_m[a:z][mask] - BANK_BASES[b]).astype(np.int16)
                rl = (dloc[a:z][mask] - w * P).astype(np.float32)
                base = (w * K) * P
                idx_flat[b, base:base + len(sl)] = sl
                rel_flat[b, base:base + len(rl)] = rl
        NIDX = PAGE * P
        S = NIDX // 16
        idxw = np.zeros((P, BANKS * NCALLB * S), np.int16)
        for b in range(BANKS):
            for j in range(NCALLB):
                seg = np.zeros(NIDX, np.int16)
                have = idx_flat[b, j * NIDX:(j + 1) * NIDX]
                seg[:len(have)] = have
                w16 = seg.reshape(S, 16).T
                col0 = (b * NCALLB + j) * S
                idxw[:, col0:col0 + S] = np.tile(w16, (CORES, 1))
        dstrel = np.full((P, NCHUNKS), -1.0, np.float32)
        for w in range(WPC):
            for b in range(BANKS):
                for k in range(K):
                    cid = (w * BANKS + b) * K + k
                    seg = rel_flat[b, (w * K + k) * P:(w * K + k + 1) * P]
                    dstrel[:, cid] = seg
        out.append({"idxw": idxw, "dstrel": dstrel})
    consts = {"K": K, "CPB": CPB, "NCALLB": NCALLB, "NCHUNKS": NCHUNKS,
              "NIDX": PAGE * P, "S": (PAGE * P) // 16}
    return out, consts


def _pack_mats(W, att_src, att_dst):
    """Weight matrix packed for the interleaved row layout.
    Returns wcat [F_in, 136] (cols h*33 zero, h*33+1+c = W col h*32+c,
    132+h = W@att_src_h) and wad [F_in, 4] (= W@att_dst)."""
    F_in = W.shape[0]
    W = W.astype(np.float32)
    wcat = np.zeros((F_in, RW), np.float32)
    for h in range(H):
        wcat[:, h * 33 + 1:(h + 1) * 33] = W[:, h * C:(h + 1) * C]
        wcat[:, 132 + h] = W[:, h * C:(h + 1) * C] @ att_src[h].astype(np.float32)
    wad = np.zeros((F_in, H), np.float32)
    for h in range(H):
        wad[:, h] = W[:, h * C:(h + 1) * C] @ att_dst[h].astype(np.float32)
    return wcat, wad


def emulate(inputs, perm, consts):
    """Numpy emulation of the device algorithm (layout-faithful, f32 math)."""
    K = consts["K"]
    x = np.asarray(inputs["x"], np.float32)
    b1 = np.asarray(inputs["b1"], np.float32)
    b2 = np.asarray(inputs["b2"], np.float32)
    wcat1, wad1 = _pack_mats(np.asarray(inputs["W1"], np.float32),
                             np.asarray(inputs["att_src1"]), np.asarray(inputs["att_dst1"]))
    wcat2, wad2 = _pack_mats(np.asarray(inputs["W2"], np.float32),
                             np.asarray(inputs["att_src2"]), np.asarray(inputs["att_dst2"]))
    brow = np.zeros(RW, np.float32)
    for h in range(H):
        brow[h * 33] = 1.0

    def layer(xin, wcat, wad, bias, concat):
        t = (xin @ wcat + brow).astype(np.float16)   # [N, 136] table
        ad = (xin @ wad).astype(np.float16)          # [N, 4]
        outs = []
        for m in range(CORES):
            pw = np.zeros((MPAD, 132), np.float64)
            idxw, dstrel = perm[m]["idxw"], perm[m]["dstrel"]
            S, NCALLB, CPB = consts["S"], consts["NCALLB"], consts["CPB"]
            for b in range(BANKS):
                for j in range(NCALLB):
                    col0 = (b * NCALLB + j) * S
                    seg = idxw[:16, col0:col0 + S].T.reshape(-1)
                    for pg in range(PAGE):
                        cglob = j * PAGE + pg
                        if cglob >= CPB:
                            break
                        w, k = cglob // K, cglob % K
                        cid = (w * BANKS + b) * K + k
                        lanes = seg[pg * P:(pg + 1) * P].astype(np.int64) + BANK_BASES[b]
                        rows = t[lanes].astype(np.float32)
                        rel = dstrel[:, cid]
                        valid = rel >= 0
                        d = np.where(valid, rel, 0).astype(np.int64) + w * P
                        e = rows[:, 132:136] + ad[m * M + np.minimum(d, M - 1)].astype(np.float32)
                        e = np.where(e >= 0, e, NEG_SLOPE * e)
                        ex = np.exp(e).astype(np.float16).astype(np.float32)
                        rhs = (rows[:, 0:132].reshape(P, H, 33)
                               * ex[:, :, None]).astype(np.float16).astype(np.float32)
                        np.add.at(pw, d, rhs.reshape(P, 132) * valid[:, None])
            pw = pw[:M]
            s = pw.reshape(M, H, 33)[:, :, 0] + EPS
            unn = pw.reshape(M, H, 33)[:, :, 1:33]
            o = unn / s[:, :, None]
            o = o.reshape(M, F) + (bias if concat else 0)
            outs.append(o.astype(np.float32))
        full = np.concatenate(outs, axis=0)
        return full

    h1 = layer(x, wcat1, wad1, b1, True)
    h1 = np.where(h1 > 0, h1, np.expm1(h1)).astype(np.float16).astype(np.float32)
    o2 = layer(h1, wcat2, wad2, None, False)
    o2 = o2.reshape(N, H, C).mean(axis=1) + b2
    return o2.astype(np.float32)


# ======================================================================
# device program (Bass/Tile)
# ======================================================================
import concourse.bacc as bacc
import concourse.bass as bass
import concourse.mybir as mybir
import concourse.tile as tile
from concourse.tile import ScopedClock
from concourse.masks import make_identity
from concourse.bass_utils import run_bass_kernel_spmd

F16 = mybir.dt.float16
F32 = mybir.dt.float32
I16 = mybir.dt.int16
AF = mybir.ActivationFunctionType
ALU = mybir.AluOpType
NGRP = NPAD // 1024          # 98 phase-0 groups of 8 tiles
NT8 = 8                      # tiles per group

# ---------------------------------------------------------------- drain patch
# walrus allows at most ONE sync wait on CTRL/DMA instructions, but the Tile
# kernel-tail drain waits on every DMA sem lane used (up to 16). Split them.
def _patched_drain_and_barrier(self, tick_clock, wait_clock):
    drain_inst = self.nc.sync.drain()
    wait_clock.add_sem_waits(
        drain_inst.ins, ScopedClock({None: tick_clock.global_clock})
    )
    si = drain_inst.ins.sync_info
    waits = list(si.on_wait or []) if si is not None else []
    if len(waits) > 1:
        si.on_wait = waits[:1]
        for w in waits[1:]:
            extra = self.nc.sync.drain()
            esi = extra.ins.sync_info
            if esi is None:
                import bass_rust
                extra.ins.sync_info = bass_rust.SyncInfo(on_wait=[], on_update=[])
                esi = extra.ins.sync_info
            esi.on_wait = [w]
    self.nc.all_engine_barrier()
    assert self.sems is not None
    popped = self.nc._tile_sem_poison_stack.pop()
    assert popped is self._sem_poison
    self.nc.clear_and_free_semaphores(list(self.sems.allocated().values()))
    self.nc.all_engine_barrier()

tile.TileContext._drain_and_barrier = _patched_drain_and_barrier


_NC_CACHE = {}


def build(consts):
    ck = tuple(sorted(consts.items()))
    if ck in _NC_CACHE:
        return _NC_CACHE[ck]
    K = consts["K"]
    CPB = consts["CPB"]
    NCALLB = consts["NCALLB"]
    NCHUNKS = consts["NCHUNKS"]
    NIDX = consts["NIDX"]
    S = consts["S"]
    CW = BANKS * K               # chunks per window (20)

    nc = bacc.Bacc("TRN2", target_bir_lowering=False, debug=False,
                   num_devices=CORES, num_swdge_queues=4)

    # ------------------------------------------------------------- tensors
    xT = nc.dram_tensor("xT", [P, NPAD], F16, kind="ExternalInput")
    xTs = nc.dram_tensor("xTs", [P, XSPAD], F16, kind="ExternalInput")
    wcat1 = nc.dram_tensor("wcat1", [P, RW], F16, kind="ExternalInput")
    wcat2 = nc.dram_tensor("wcat2", [P, RW], F16, kind="ExternalInput")
    wad1 = nc.dram_tensor("wad1", [P, H], F16, kind="ExternalInput")
    wad2 = nc.dram_tensor("wad2", [P, H], F16, kind="ExternalInput")
    brow = nc.dram_tensor("brow", [1, RW], F16, kind="ExternalInput")
    b1t = nc.dram_tensor("b1t", [P, F], F32, kind="ExternalInput")
    b2t = nc.dram_tensor("b2t", [P, C], F32, kind="ExternalInput")
    idxw = nc.dram_tensor("idxw", [P, BANKS * NCALLB * S], I16, kind="ExternalInput")
    dstrel = nc.dram_tensor("dstrel", [P, NCHUNKS], F32, kind="ExternalInput")
    out2 = nc.dram_tensor("out2", [MPAD, C], F32, kind="ExternalOutput")

    table = [nc.dram_tensor(f"table{l}", [NPAD, ROW], F16) for l in (1, 2)]
    h_shard = nc.dram_tensor("h_shard", [P, MPAD], F16)
    h_full = nc.dram_tensor("h_full", [CORES, P, MPAD], F16, addr_space="Shared")

    with tile.TileContext(nc) as tc:
        with (
            tc.tile_pool(name="const", bufs=1) as cpool,
            tc.tile_pool(name="resident", bufs=1) as rpool,
            tc.tile_pool(name="p0", bufs=4) as p0pool,
            tc.tile_pool(name="p0st", bufs=4) as p0st,
            tc.tile_pool(name="p0ps", bufs=2, space="PSUM") as p0ps,
            tc.tile_pool(name="gat", bufs=4) as gpool,
            tc.tile_pool(name="oh", bufs=2 * CW + 4) as ohpool,
            tc.tile_pool(name="cmp", bufs=5) as cmppool,
            tc.tile_pool(name="wps", bufs=2, space="PSUM") as wps,
            tc.tile_pool(name="eps", bufs=2, space="PSUM") as epsp,
            tc.tile_pool(name="trps", bufs=2, space="PSUM") as trps,
            tc.tile_pool(name="p3", bufs=4) as p3pool,
            tc.tile_pool(name="stg", bufs=2) as stgpool,
        ):
            # ---------------- constants
            ident = cpool.tile([P, P], F16)
            make_identity(nc, ident[:])
            iota_i = cpool.tile([P, P], mybir.dt.int32)
            nc.gpsimd.iota(iota_i[:], pattern=[[1, P]], base=0, channel_multiplier=0)
            iota_row = cpool.tile([P, P], F16)
            nc.vector.tensor_copy(iota_row[:], iota_i[:])
            ones_row = cpool.tile([1, P], F16)
            nc.vector.memset(ones_row[:], 1.0)

            wc = []
            for l, t in ((0, wcat1), (1, wcat2)):
                w_t = cpool.tile([P, RW], F16, tag=f"wc{l}")
                nc.sync.dma_start(out=w_t[:], in_=t[:, :])
                wc.append(w_t)
            wad = []
            for l, t in ((0, wad1), (1, wad2)):
                w_t = cpool.tile([P, H], F16, tag=f"wad{l}")
                nc.sync.dma_start(out=w_t[:], in_=t[:, :])
                wad.append(w_t)
            brow_t = cpool.tile([1, RW], F16)
            nc.sync.dma_start(out=brow_t[:], in_=brow[:, :])
            b1_t = cpool.tile([P, F], F32)
            nc.sync.dma_start(out=b1_t[:], in_=b1t[:, :])
            b2_t = cpool.tile([P, C], F32)
            nc.sync.dma_start(out=b2_t[:], in_=b2t[:, :])

            idx_t = rpool.tile([P, BANKS * NCALLB * S], I16)
            nc.sync.dma_start(out=idx_t[:], in_=idxw[:, :])
            rel_t = rpool.tile([P, NCHUNKS], F32)
            nc.sync.dma_start(out=rel_t[:], in_=dstrel[:, :])


            adres0 = rpool.tile([P, WPC * H], F16, tag="ad0")
            adres1 = rpool.tile([P, WPC * H], F16, tag="ad1")
            adres = [adres0, adres1]

            # layer-1 a_dst from the local xT shard (batched loads)
            for g in range(13):
                wlo = g * 8
                nwin = min(8, WPC - wlo)
                if nwin <= 0:
                    break
                lt = p0pool.tile([P, 1024], F16, tag="adl")
                nc.sync.dma_start(out=lt[:], in_=xTs[:, g * 1024:(g + 1) * 1024])
                aps = epsp.tile([P, CW * H], F32, tag="ep")
                for k in range(nwin):
                    nc.tensor.matmul(aps[:, k * H:(k + 1) * H],
                                     lhsT=lt[:, k * P:(k + 1) * P], rhs=wad[0][:],
                                     start=True, stop=True)
                nc.vector.tensor_copy(adres0[:, wlo * H:(wlo + nwin) * H],
                                      aps[:, 0:nwin * H])

            for L in range(2):
                # ======================================================= phase 0
                for g in range(NGRP):
                    lt = p0pool.tile([P, 1024], F16, tag="p0l")
                    if L == 0:
                        nc.sync.dma_start(out=lt[:], in_=xT[:, g * 1024:(g + 1) * 1024])
                    else:
                        # table rows are REAL-node indexed; h_full blocks hold
                        # 12500 real cols (+44 pad) each — split at 12500s.
                        n0 = g * 1024
                        done = 0
                        while done < 1024:
                            nr = n0 + done
                            if nr >= N:
                                nc.vector.memset(lt[:, done:1024], 0.0)
                                break
                            blk = nr // M
                            off = nr % M
                            take = min(1024 - done, M - off, N - nr)
                            nc.sync.dma_start(
                                out=lt[:, done:done + take],
                                in_=h_full[blk, :, off:off + take])
                            done += take
                    stage = p0st.tile([P, NT8, RW], F16, tag="st")
                    # 8 matmul pairs into 3-wide PSUM tiles (bank limit 2KB);
                    # batched PSUM->SBUF f16 copies on the scalar engine
                    for k0 in (0, 3, 6):
                        kn = min(3, NT8 - k0)
                        ps = p0ps.tile([P, 3, RW], F32, tag="p0p")
                        for kk in range(kn):
                            k = k0 + kk
                            nc.tensor.matmul(ps[:, kk, :],
                                             lhsT=lt[:, k * P:(k + 1) * P],
                                             rhs=wc[L][:], start=True, stop=False)
                            nc.tensor.matmul(ps[:, kk, :], lhsT=ones_row[:1, :],
                                             rhs=brow_t[:], start=False, stop=True)
                        nc.scalar.activation(stage[:, k0:k0 + kn, :],
                                             ps[:, 0:kn, :], AF.Copy)
                    nc.sync.dma_start(
                        out=table[L][g * 1024:(g + 1) * 1024, 0:RW].rearrange(
                            "(k p) c -> p k c", p=P),
                        in_=stage[:])

                # ======================================================= edges
                # Software-pipelined by one window: phase 3 of window w-1 is
                # emitted between pass A and pass B of window w, so the vector
                # engine never stalls on the PE/Act round trips of phase 3.
                nextcall = [0] * BANKS
                gtiles = [dict() for _ in range(BANKS)]
                stg = {"h": None, "o": None}

                def pass_a(w):
                    for b in range(BANKS):
                        while nextcall[b] * PAGE < min((w + 1) * K, CPB):
                            j = nextcall[b]
                            gt = gpool.tile([P, PAGE, ROW], F16, tag=f"g{b}")
                            col0 = (b * NCALLB + j) * S
                            nc.gpsimd.dma_gather(
                                gt[:], table[L][BANK_BASES[b]:BANK_BASES[b] + BANK_ROWS, :],
                                idx_t[:, col0:col0 + S], NIDX, NIDX, ROW,
                                queue_num=b)
                            gtiles[b][j] = gt
                            if j - 2 in gtiles[b]:
                                del gtiles[b][j - 2]
                            nextcall[b] += 1
                    eps = epsp.tile([P, CW * H], F32, tag="ep")
                    ohs = []
                    for cw in range(CW):
                        cid = (w * BANKS + (cw // K)) * K + (cw % K)
                        oh = ohpool.tile([P, P], F16, tag="oh")
                        eng = nc.gpsimd if cw >= CW - K else nc.vector
                        eng.tensor_scalar(
                            oh[:], iota_row[:], rel_t[:, cid:cid + 1], None,
                            op0=ALU.is_equal)
                        ohs.append(oh)
                    ohTs = []
                    for b in range(BANKS):
                        trp = trps.tile([P, K * P], F16, tag="tr")
                        for k in range(K):
                            nc.tensor.transpose(trp[:, k * P:(k + 1) * P],
                                                ohs[b * K + k][:], ident[:])
                        ohT = cmppool.tile([P, K * P], F16, tag="ohT")
                        nc.scalar.activation(ohT[:], trp[:], AF.Copy)
                        ohTs.append(ohT)
                    # paired open/close per chunk: PSUM tracks only one open
                    # accumulation group per bank, so each chunk's two matmuls
                    # (ohT.T@adres then I.T@rows_as) must be adjacent.
                    for cw in range(CW):
                        b, k = cw // K, cw % K
                        cglob = w * K + k
                        gt = gtiles[b][cglob // PAGE]
                        nc.tensor.matmul(eps[:, cw * H:(cw + 1) * H],
                                         lhsT=ohTs[b][:, k * P:(k + 1) * P],
                                         rhs=adres[L][:, w * H:(w + 1) * H],
                                         start=True, stop=False)
                        nc.tensor.matmul(eps[:, cw * H:(cw + 1) * H],
                                         lhsT=ident[:],
                                         rhs=gt[:, cglob % PAGE, 132:136],
                                         start=False, stop=True)
                    return eps, ohs

                def pass_b(w, eps, ohs):
                    # exp(lrelu(e)) = max(exp(e), exp(0.2 e)) since exp is
                    # monotonic: two scalar-engine exps + one f16 vector max
                    exa = p3pool.tile([P, CW * H], F16, tag="exa")
                    nc.scalar.activation(exa[:], eps[:], AF.Exp)
                    exb = p3pool.tile([P, CW * H], F16, tag="exb")
                    nc.scalar.activation(exb[:], eps[:], AF.Exp, scale=NEG_SLOPE)
                    ex = p3pool.tile([P, CW * H], F16, tag="ex")
                    nc.vector.tensor_tensor(ex[:], exa[:], exb[:], op=ALU.max)
                    pw = wps.tile([P, RW], F32, tag="pw")
                    # fused rhs over runs of chunks sharing one gather tile
                    for b in range(BANKS):
                        k = 0
                        while k < K:
                            cglob = w * K + k
                            j = cglob // PAGE
                            pg = cglob % PAGE
                            kn = min(K - k, PAGE - pg)
                            cw0 = b * K + k
                            rows3 = gtiles[b][j][:, pg:pg + kn, 0:132]
                            rhs = cmppool.tile([P, K, 132], F16, tag="rhs")
                            nc.vector.tensor_tensor(
                                rhs[:, 0:kn, :].rearrange("p k (h c) -> p k h c", h=H),
                                rows3.rearrange("p k (h c) -> p k h c", h=H),
                                ex[:, cw0 * H:(cw0 + kn) * H].rearrange(
                                    "p (k h) -> p k h", h=H)[:, :, :, None]
                                    .to_broadcast([P, kn, H, 33]),
                                op=ALU.mult)
                            for kk in range(kn):
                                cw = cw0 + kk
                                nc.tensor.matmul(pw[:, 0:132], lhsT=ohs[cw][:],
                                                 rhs=rhs[:, kk, :],
                                                 start=(cw == 0), stop=(cw == CW - 1))
                            k += kn
                    return pw

                def phase3(w, pw):
                    sp = p3pool.tile([P, H], F32, tag="s")
                    nc.scalar.activation(
                        sp[:, :, None],
                        pw[:, 0:132].rearrange("p (h c) -> p h c", h=H)[:, :, 0:1],
                        AF.Copy, bias=EPS)
                    r = p3pool.tile([P, H], F32, tag="r")
                    nc.vector.reciprocal(r[:], sp[:])
                    hw = p3pool.tile([P, F], F32, tag="hw")
                    nc.vector.tensor_tensor(
                        hw[:].rearrange("p (h c) -> p h c", h=H),
                        pw[:, 0:132].rearrange("p (h c) -> p h c", h=H)[:, :, 1:33],
                        r[:, :, None].to_broadcast([P, H, C]),
                        op=ALU.mult)
                    if L == 0:
                        if w % 8 == 0:
                            hs_new = stgpool.tile([P, 1024], F16, tag="hs")
                            stg["h"] = hs_new
                        hstage = stg["h"]
                        nc.vector.tensor_tensor(hw[:], hw[:], b1_t[:], op=ALU.add)
                        # elu
                        mn = p3pool.tile([P, F], F32, tag="mn")
                        nc.vector.tensor_scalar(mn[:], hw[:], 0.0, None, op0=ALU.min)
                        mx = p3pool.tile([P, F], F32, tag="mx")
                        nc.vector.tensor_scalar(mx[:], hw[:], 0.0, None, op0=ALU.max)
                        ek = p3pool.tile([P, F], F32, tag="ek")
                        nc.scalar.activation(ek[:], mn[:], AF.Exp)
                        he = p3pool.tile([P, F], F16, tag="he")
                        nc.vector.scalar_tensor_tensor(
                            he[:], ek[:], -1.0, mx[:], op0=ALU.add, op1=ALU.add)
                        trp = trps.tile([P, K * P], F16, tag="tr")
                        nc.tensor.transpose(trp[:, 0:P], he[:], ident[:])
                        hcol = (w % 8) * P
                        nc.scalar.activation(hstage[:, hcol:hcol + P], trp[:, 0:P],
                                             AF.Copy)
                        adp = epsp.tile([P, CW * H], F32, tag="ep")
                        nc.tensor.matmul(adp[:, 0:H],
                                         lhsT=hstage[:, hcol:hcol + P], rhs=wad[1][:],
                                         start=True, stop=True)
                        nc.vector.tensor_copy(adres1[:, w * H:(w + 1) * H], adp[:, 0:H])
                        if w % 8 == 7 or w == WPC - 1:
                            g0 = (w // 8) * 1024
                            gw = min(1024, MPAD - g0)
                            nc.sync.dma_start(out=h_shard[:, g0:g0 + gw],
                                              in_=hstage[:, 0:gw])
                    else:
                        if w % 8 == 0:
                            os_new = stgpool.tile([P, NT8, C], F32, tag="os")
                            stg["o"] = os_new
                        ostage = stg["o"]
                        red = p3pool.tile([P, C], F32, tag="red")
                        nc.vector.tensor_reduce(
                            red[:], hw[:].rearrange("p (h c) -> p c h", h=H),
                            axis=mybir.AxisListType.X, op=ALU.add)
                        nc.vector.scalar_tensor_tensor(
                            ostage[:, w % 8, :], red[:], 1.0 / H, b2_t[:],
                            op0=ALU.mult, op1=ALU.add)
                        if w % 8 == 7 or w == WPC - 1:
                            g0 = (w // 8) * 1024
                            kw = (w % 8) + 1
                            nc.sync.dma_start(
                                out=out2[g0:g0 + kw * P, :].rearrange(
                                    "(k p) c -> p k c", p=P),
                                in_=ostage[:, 0:kw, :])

                prev = None
                for w in range(WPC):
                    eps, ohs = pass_a(w)
                    if prev is not None:
                        phase3(w - 1, prev)
                    prev = pass_b(w, eps, ohs)
                phase3(WPC - 1, prev)

                if L == 0:
                    nc.gpsimd.collective_compute(
                        "AllGather", ALU.bypass,
                        replica_groups=[list(range(CORES))],
                        ins=[h_shard.ap()],
                        outs=[h_full.ap()],
                    )
    nc.compile()
    _NC_CACHE[ck] = nc
    return nc


def make_inmaps(inputs, perm, consts):
    x = np.asarray(inputs["x"], np.float32)
    wcat1, wad1 = _pack_mats(np.asarray(inputs["W1"], np.float32),
                             np.asarray(inputs["att_src1"]), np.asarray(inputs["att_dst1"]))
    wcat2, wad2 = _pack_mats(np.asarray(inputs["W2"], np.float32),
                             np.asarray(inputs["att_src2"]), np.asarray(inputs["att_dst2"]))
    b1 = np.asarray(inputs["b1"], np.float32)
    b2 = np.asarray(inputs["b2"], np.float32)
    brow_np = np.zeros((1, RW), np.float16)
    for h in range(H):
        brow_np[0, h * 33] = 1.0
    xT_np = np.zeros((P, NPAD), np.float16)
    xT_np[:, :N] = np.ascontiguousarray(x.T).astype(np.float16)
    common = {
        "xT": xT_np,
        "wcat1": wcat1.astype(np.float16), "wcat2": wcat2.astype(np.float16),
        "wad1": wad1.astype(np.float16), "wad2": wad2.astype(np.float16),
        "brow": brow_np,
        "b1t": np.tile(b1[None, :], (P, 1)).astype(np.float32),
        "b2t": np.tile(b2[None, :], (P, 1)).astype(np.float32),
    }
    maps = []
    for m in range(CORES):
        im = dict(common)
        xs = np.zeros((P, XSPAD), np.float16)
        xs[:, :M] = xT_np[:, m * M:(m + 1) * M]
        im["xTs"] = xs
        im["idxw"] = perm[m]["idxw"]
        im["dstrel"] = perm[m]["dstrel"]
        maps.append(im)
    return maps


def run_on_hw(inputs, perm, consts):
    nc = build(consts)
    maps = make_inmaps(inputs, perm, consts)
    res = run_bass_kernel_spmd(nc, maps, core_ids=list(range(CORES)))
    return np.concatenate([res.results[m]["out2"][:M] for m in range(CORES)], axis=0)


def kernel(**inputs):
    perm, consts = _host_prep(np.asarray(inputs["edge_index"]))
    out = run_on_hw(inputs, perm, consts)
    if not np.isfinite(out).all():
        # transient first-dispatch flakiness: retry once
        out = run_on_hw(inputs, perm, consts)
    return out


# revision 26
# speedup vs baseline: 1.1101x; 1.0111x over previous
"""GAT 2-layer kernel for 8 Trainium2 NeuronCores (v2).

Strategy (dst-sharded edge partitioning, engine-balanced):
  - Nodes and their in-edges sharded by dst across 8 cores (12500 nodes each).
    Self-loops appended as regular edges; edges sorted by dst window (98
    windows of 128 dsts), grouped into 4 src-banks (int16-indexable 32768-row
    overlapping bank views of the node table), chunk-padded to K=5 chunks of
    128 edges per (window, bank).
  - Per-node table rows [(1|xh_h*32) x4 | a_src(4) | pad] (512B stride) are
    computed on-device with dense matmuls (8 tiles per DMA, batched stores via
    a 3D DRAM access pattern); per-edge rows fetched with the gpsimd
    dma_gather custom op (int16 indices, bank-relative, 4 SWDGE queues).
  - e = a_src[src] + a_dst[dst] accumulated in PSUM per chunk by two matmuls:
    ohT.T @ a_dst_window (ohT = PE-transposed one-hot) + I.T @ rows[132:136].
    leaky-relu as one fused scalar_tensor_tensor; exp on the scalar engine.
  - Softmax without max-subtraction, normalization after aggregation:
    out[d] = (sum ex*xh) / (sum ex).  The interleaved (1|xh) row layout makes
    rhs = rows * ex_broadcast a single fused vector op; the leading-ones
    columns accumulate sum(ex) in the same scatter matmul.
  - Scatter within a window is a one-hot matmul into PSUM over 20 chunks.
  - Biases applied after normalization (phase 3); between layers an AllGather
    of fp16 transposed h shards feeds layer 2's table build.
"""

import numpy as np

# ---------------------------------------------------------------- constants
N = 100000
E_IN = 1600000
CORES = 8
M = N // CORES              # 12500 nodes per core
P = 128
WPC = (M + P - 1) // P      # 98 windows per core
H, C = 4, 32                # heads x channels (both layers)
F = 128                     # feature width (= H*C)
ROW = 256                   # f16 elements per table row (512B)
RW = 136                    # used columns: 4*(1+32) + 4 a_src
BANKS = 4
BANK_ROWS = 32768
BANK_BASES = [0, 22411, 44822, 67232]
PAGE = 8                    # chunks per dma_gather call (1024 idx)
NEG_SLOPE = 0.2
EPS = 1e-30

NPAD = 100352               # N padded to 8*12544 (= 98 groups of 1024)
MPAD = 12544                # per-core padded node count (98*128)
XSPAD = 13312               # xTs padded to 13*1024


def _host_prep(edge_index):
    """Partition/sort/bank/pad the edge structure. Returns per-core data:
      idxw   [128, BANKS*NCALLB*S] int16 -- wrapped bank-relative gather idx
      dstrel [128, NCHUNKS]        f32   -- dst lane relative to window (-1 pad)
    """
    src = np.concatenate([edge_index[0], np.arange(N, dtype=np.int64)]).astype(np.int64)
    dst = np.concatenate([edge_index[1], np.arange(N, dtype=np.int64)]).astype(np.int64)

    bases = np.asarray(BANK_BASES, np.int64)
    cores = []
    maxK = 0
    for m in range(CORES):
        sel = (dst // M) == m
        s_m = src[sel]
        dloc = (dst[sel] - m * M).astype(np.int64)
        win = dloc // P
        order = np.argsort(win, kind="stable")
        s_m, dloc, win = s_m[order], dloc[order], win[order]
        hi_b = np.searchsorted(bases, s_m, side="right") - 1
        lo_ok = (hi_b > 0) & (s_m < bases[np.maximum(hi_b - 1, 0)] + BANK_ROWS)
        lo_b = np.where(lo_ok, hi_b - 1, hi_b)
        bank = np.empty(len(s_m), np.int8)
        wstarts = np.searchsorted(win, np.arange(WPC + 1))
        for w in range(WPC):
            a, z = wstarts[w], wstarts[w + 1]
            nb = z - a
            T = -(-nb // BANKS)
            cnt = np.bincount(hi_b[a:z][~lo_ok[a:z]], minlength=BANKS).astype(np.int64)
            bw = hi_b[a:z].copy()
            rigid = ~lo_ok[a:z]
            bw[rigid] = hi_b[a:z][rigid]
            for pnr in range(BANKS - 1):
                fm = lo_ok[a:z] & (lo_b[a:z] == pnr)
                f = int(fm.sum())
                give = min(f, max(0, T - int(cnt[pnr])))
                idxs = np.flatnonzero(fm)
                bw[idxs[:give]] = pnr
                bw[idxs[give:]] = pnr + 1
                cnt[pnr] += give
                cnt[pnr + 1] += f - give
            bank[a:z] = bw
            maxK = max(maxK, int(-(-cnt.max() // P)))
        cores.append((s_m, dloc, win, bank, wstarts))

    K = max(5, maxK)
    CPB = WPC * K
    NCALLB = (CPB + PAGE - 1) // PAGE
    NCHUNKS = WPC * BANKS * K

    out = []
    for m in range(CORES):
        s_m, dloc, win, bank, wstarts = cores[m]
        idx_flat = np.zeros((BANKS, CPB * P), np.int16)
        rel_flat = np.full((BANKS, CPB * P), -1.0, np.float32)
        for w in range(WPC):
            a, z = wstarts[w], wstarts[w + 1]
            bw = bank[a:z]
            for b in range(BANKS):
                mask = bw == b
                sl = (s_m[a:z][mask] - BANK_BASES[b]).astype(np.int16)
                rl = (dloc[a:z][mask] - w * P).astype(np.float32)
                base = (w * K) * P
                idx_flat[b, base:base + len(sl)] = sl
                rel_flat[b, base:base + len(rl)] = rl
        NIDX = PAGE * P
        S = NIDX // 16
        idxw = np.zeros((P, BANKS * NCALLB * S), np.int16)
        for b in range(BANKS):
            for j in range(NCALLB):
                seg = np.zeros(NIDX, np.int16)
                have = idx_flat[b, j * NIDX:(j + 1) * NIDX]
                seg[:len(have)] = have
                w16 = seg.reshape(S, 16).T
                col0 = (b * NCALLB + j) * S
                idxw[:, col0:col0 + S] = np.tile(w16, (CORES, 1))
        dstrel = np.full((P, NCHUNKS), -1.0, np.float32)
        for w in range(WPC):
            for b in range(BANKS):
                for k in range(K):
                    cid = (w * BANKS + b) * K + k
                    seg = rel_flat[b, (w * K + k) * P:(w * K + k + 1) * P]
                    dstrel[:, cid] = seg
        out.append({"idxw": idxw, "dstrel": dstrel})
    consts = {"K": K, "CPB": CPB, "NCALLB": NCALLB, "NCHUNKS": NCHUNKS,
              "NIDX": PAGE * P, "S": (PAGE * P) // 16}
    return out, consts


def _pack_mats(W, att_src, att_dst):
    """Weight matrix packed for the interleaved row layout.
    Returns wcat [F_in, 136] (cols h*33 zero, h*33+1+c = W col h*32+c,
    132+h = W@att_src_h) and wad [F_in, 4] (= W@att_dst)."""
    F_in = W.shape[0]
    W = W.astype(np.float32)
    wcat = np.zeros((F_in, RW), np.float32)
    for h in range(H):
        wcat[:, h * 33 + 1:(h + 1) * 33] = W[:, h * C:(h + 1) * C]
        wcat[:, 132 + h] = W[:, h * C:(h + 1) * C] @ att_src[h].astype(np.float32)
    wad = np.zeros((F_in, H), np.float32)
    for h in range(H):
        wad[:, h] = W[:, h * C:(h + 1) * C] @ att_dst[h].astype(np.float32)
    return wcat, wad


def emulate(inputs, perm, consts):
    """Numpy emulation of the device algorithm (layout-faithful, f32 math)."""
    K = consts["K"]
    x = np.asarray(inputs["x"], np.float32)
    b1 = np.asarray(inputs["b1"], np.float32)
    b2 = np.asarray(inputs["b2"], np.float32)
    wcat1, wad1 = _pack_mats(np.asarray(inputs["W1"], np.float32),
                             np.asarray(inputs["att_src1"]), np.asarray(inputs["att_dst1"]))
    wcat2, wad2 = _pack_mats(np.asarray(inputs["W2"], np.float32),
                             np.asarray(inputs["att_src2"]), np.asarray(inputs["att_dst2"]))
    brow = np.zeros(RW, np.float32)
    for h in range(H):
        brow[h * 33] = 1.0

    def layer(xin, wcat, wad, bias, concat):
        t = (xin @ wcat + brow).astype(np.float16)   # [N, 136] table
        ad = (xin @ wad).astype(np.float16)          # [N, 4]
        outs = []
        for m in range(CORES):
            pw = np.zeros((MPAD, 132), np.float64)
            idxw, dstrel = perm[m]["idxw"], perm[m]["dstrel"]
            S, NCALLB, CPB = consts["S"], consts["NCALLB"], consts["CPB"]
            for b in range(BANKS):
                for j in range(NCALLB):
                    col0 = (b * NCALLB + j) * S
                    seg = idxw[:16, col0:col0 + S].T.reshape(-1)
                    for pg in range(PAGE):
                        cglob = j * PAGE + pg
                        if cglob >= CPB:
                            break
                        w, k = cglob // K, cglob % K
                        cid = (w * BANKS + b) * K + k
                        lanes = seg[pg * P:(pg + 1) * P].astype(np.int64) + BANK_BASES[b]
                        rows = t[lanes].astype(np.float32)
                        rel = dstrel[:, cid]
                        valid = rel >= 0
                        d = np.where(valid, rel, 0).astype(np.int64) + w * P
                        e = rows[:, 132:136] + ad[m * M + np.minimum(d, M - 1)].astype(np.float32)
                        e = np.where(e >= 0, e, NEG_SLOPE * e)
                        ex = np.exp(e).astype(np.float16).astype(np.float32)
                        rhs = (rows[:, 0:132].reshape(P, H, 33)
                               * ex[:, :, None]).astype(np.float16).astype(np.float32)
                        np.add.at(pw, d, rhs.reshape(P, 132) * valid[:, None])
            pw = pw[:M]
            s = pw.reshape(M, H, 33)[:, :, 0] + EPS
            unn = pw.reshape(M, H, 33)[:, :, 1:33]
            o = unn / s[:, :, None]
            o = o.reshape(M, F) + (bias if concat else 0)
            outs.append(o.astype(np.float32))
        full = np.concatenate(outs, axis=0)
        return full

    h1 = layer(x, wcat1, wad1, b1, True)
    h1 = np.where(h1 > 0, h1, np.expm1(h1)).astype(np.float16).astype(np.float32)
    o2 = layer(h1, wcat2, wad2, None, False)
    o2 = o2.reshape(N, H, C).mean(axis=1) + b2
    return o2.astype(np.float32)


# ======================================================================
# device program (Bass/Tile)
# ======================================================================
import concourse.bacc as bacc
import concourse.bass as bass
import concourse.mybir as mybir
import concourse.tile as tile
from concourse.tile import ScopedClock
from concourse.masks import make_identity
from concourse.bass_utils import run_bass_kernel_spmd

F16 = mybir.dt.float16
F32 = mybir.dt.float32
I16 = mybir.dt.int16
AF = mybir.ActivationFunctionType
ALU = mybir.AluOpType
NGRP = NPAD // 1024          # 98 phase-0 groups of 8 tiles
NT8 = 8                      # tiles per group

# ---------------------------------------------------------------- drain patch
# walrus allows at most ONE sync wait on CTRL/DMA instructions, but the Tile
# kernel-tail drain waits on every DMA sem lane used (up to 16). Split them.
def _patched_drain_and_barrier(self, tick_clock, wait_clock):
    drain_inst = self.nc.sync.drain()
    wait_clock.add_sem_waits(
        drain_inst.ins, ScopedClock({None: tick_clock.global_clock})
    )
    si = drain_inst.ins.sync_info
    waits = list(si.on_wait or []) if si is not None else []
    if len(waits) > 1:
        si.on_wait = waits[:1]
        for w in waits[1:]:
            extra = self.nc.sync.drain()
            esi = extra.ins.sync_info
            if esi is None:
                import bass_rust
                extra.ins.sync_info = bass_rust.SyncInfo(on_wait=[], on_update=[])
                esi = extra.ins.sync_info
            esi.on_wait = [w]
    self.nc.all_engine_barrier()
    assert self.sems is not None
    popped = self.nc._tile_sem_poison_stack.pop()
    assert popped is self._sem_poison
    self.nc.clear_and_free_semaphores(list(self.sems.allocated().values()))
    self.nc.all_engine_barrier()

tile.TileContext._drain_and_barrier = _patched_drain_and_barrier


_NC_CACHE = {}


def build(consts):
    ck = tuple(sorted(consts.items()))
    if ck in _NC_CACHE:
        return _NC_CACHE[ck]
    K = consts["K"]
    CPB = consts["CPB"]
    NCALLB = consts["NCALLB"]
    NCHUNKS = consts["NCHUNKS"]
    NIDX = consts["NIDX"]
    S = consts["S"]
    CW = BANKS * K               # chunks per window (20)

    nc = bacc.Bacc("TRN2", target_bir_lowering=False, debug=False,
                   num_devices=CORES, num_swdge_queues=4)

    # ------------------------------------------------------------- tensors
    xT = nc.dram_tensor("xT", [P, NPAD], F16, kind="ExternalInput")
    xTs = nc.dram_tensor("xTs", [P, XSPAD], F16, kind="ExternalInput")
    wcat1 = nc.dram_tensor("wcat1", [P, RW], F16, kind="ExternalInput")
    wcat2 = nc.dram_tensor("wcat2", [P, RW], F16, kind="ExternalInput")
    wad1 = nc.dram_tensor("wad1", [P, H], F16, kind="ExternalInput")
    wad2 = nc.dram_tensor("wad2", [P, H], F16, kind="ExternalInput")
    brow = nc.dram_tensor("brow", [1, RW], F16, kind="ExternalInput")
    b1t = nc.dram_tensor("b1t", [P, F], F32, kind="ExternalInput")
    b2t = nc.dram_tensor("b2t", [P, C], F32, kind="ExternalInput")
    idxw = nc.dram_tensor("idxw", [P, BANKS * NCALLB * S], I16, kind="ExternalInput")
    dstrel = nc.dram_tensor("dstrel", [P, NCHUNKS], F32, kind="ExternalInput")
    out2 = nc.dram_tensor("out2", [MPAD, C], F32, kind="ExternalOutput")

    table = [nc.dram_tensor(f"table{l}", [NPAD, ROW], F16) for l in (1, 2)]
    h_shard = nc.dram_tensor("h_shard", [P, MPAD], F16)
    h_full = nc.dram_tensor("h_full", [CORES, P, MPAD], F16, addr_space="Shared")

    with tile.TileContext(nc) as tc:
        with (
            tc.tile_pool(name="const", bufs=1) as cpool,
            tc.tile_pool(name="resident", bufs=1) as rpool,
            tc.tile_pool(name="p0", bufs=4) as p0pool,
            tc.tile_pool(name="p0st", bufs=4) as p0st,
            tc.tile_pool(name="p0ps", bufs=2, space="PSUM") as p0ps,
            tc.tile_pool(name="gat", bufs=4) as gpool,
            tc.tile_pool(name="oh", bufs=2 * CW + 4) as ohpool,
            tc.tile_pool(name="cmp", bufs=5) as cmppool,
            tc.tile_pool(name="wps", bufs=2, space="PSUM") as wps,
            tc.tile_pool(name="eps", bufs=2, space="PSUM") as epsp,
            tc.tile_pool(name="trps", bufs=2, space="PSUM") as trps,
            tc.tile_pool(name="p3", bufs=4) as p3pool,
            tc.tile_pool(name="stg", bufs=2) as stgpool,
        ):
            # ---------------- constants
            ident = cpool.tile([P, P], F16)
            make_identity(nc, ident[:])
            iota_i = cpool.tile([P, P], mybir.dt.int32)
            nc.gpsimd.iota(iota_i[:], pattern=[[1, P]], base=0, channel_multiplier=0)
            iota_row = cpool.tile([P, P], F16)
            nc.vector.tensor_copy(iota_row[:], iota_i[:])
            ones_row = cpool.tile([1, P], F16)
            nc.vector.memset(ones_row[:], 1.0)

            wc = []
            for l, t in ((0, wcat1), (1, wcat2)):
                w_t = cpool.tile([P, RW], F16, tag=f"wc{l}")
                nc.sync.dma_start(out=w_t[:], in_=t[:, :])
                wc.append(w_t)
            wad = []
            for l, t in ((0, wad1), (1, wad2)):
                w_t = cpool.tile([P, H], F16, tag=f"wad{l}")
                nc.sync.dma_start(out=w_t[:], in_=t[:, :])
                wad.append(w_t)
            brow_t = cpool.tile([1, RW], F16)
            nc.sync.dma_start(out=brow_t[:], in_=brow[:, :])
            b1_t = cpool.tile([P, F], F32)
            nc.sync.dma_start(out=b1_t[:], in_=b1t[:, :])
            b2_t = cpool.tile([P, C], F32)
            nc.sync.dma_start(out=b2_t[:], in_=b2t[:, :])

            idx_t = rpool.tile([P, BANKS * NCALLB * S], I16)
            nc.sync.dma_start(out=idx_t[:], in_=idxw[:, :])
            rel_t = rpool.tile([P, NCHUNKS], F32)
            nc.sync.dma_start(out=rel_t[:], in_=dstrel[:, :])


            adres0 = rpool.tile([P, WPC * H], F16, tag="ad0")
            adres1 = rpool.tile([P, WPC * H], F16, tag="ad1")
            adres = [adres0, adres1]

            # layer-1 a_dst from the local xT shard (batched loads)
            for g in range(13):
                wlo = g * 8
                nwin = min(8, WPC - wlo)
                if nwin <= 0:
                    break
                lt = p0pool.tile([P, 1024], F16, tag="adl")
                nc.sync.dma_start(out=lt[:], in_=xTs[:, g * 1024:(g + 1) * 1024])
                aps = epsp.tile([P, CW * H], F32, tag="ep")
                for k in range(nwin):
                    nc.tensor.matmul(aps[:, k * H:(k + 1) * H],
                                     lhsT=lt[:, k * P:(k + 1) * P], rhs=wad[0][:],
                                     start=True, stop=True)
                nc.vector.tensor_copy(adres0[:, wlo * H:(wlo + nwin) * H],
                                      aps[:, 0:nwin * H])

            for L in range(2):
                # ======================================================= phase 0
                for g in range(NGRP):
                    lt = p0pool.tile([P, 1024], F16, tag="p0l")
                    if L == 0:
                        nc.sync.dma_start(out=lt[:], in_=xT[:, g * 1024:(g + 1) * 1024])
                    else:
                        # table rows are REAL-node indexed; h_full blocks hold
                        # 12500 real cols (+44 pad) each — split at 12500s.
                        n0 = g * 1024
                        done = 0
                        while done < 1024:
                            nr = n0 + done
                            if nr >= N:
                                nc.vector.memset(lt[:, done:1024], 0.0)
                                break
                            blk = nr // M
                            off = nr % M
                            take = min(1024 - done, M - off, N - nr)
                            nc.sync.dma_start(
                                out=lt[:, done:done + take],
                                in_=h_full[blk, :, off:off + take])
                            done += take
                    stage = p0st.tile([P, NT8, RW], F16, tag="st")
                    # 8 matmul pairs into 3-wide PSUM tiles (bank limit 2KB);
                    # batched PSUM->SBUF f16 copies on the scalar engine
                    for k0 in (0, 3, 6):
                        kn = min(3, NT8 - k0)
                        ps = p0ps.tile([P, 3, RW], F32, tag="p0p")
                        for kk in range(kn):
                            k = k0 + kk
                            nc.tensor.matmul(ps[:, kk, :],
                                             lhsT=lt[:, k * P:(k + 1) * P],
                                             rhs=wc[L][:], start=True, stop=False)
                            nc.tensor.matmul(ps[:, kk, :], lhsT=ones_row[:1, :],
                                             rhs=brow_t[:], start=False, stop=True)
                        nc.scalar.activation(stage[:, k0:k0 + kn, :],
                                             ps[:, 0:kn, :], AF.Copy)
                    nc.sync.dma_start(
                        out=table[L][g * 1024:(g + 1) * 1024, 0:RW].rearrange(
                            "(k p) c -> p k c", p=P),
                        in_=stage[:])

                # ======================================================= edges
                # Software-pipelined by one window: phase 3 of window w-1 is
                # emitted between pass A and pass B of window w, so the vector
                # engine never stalls on the PE/Act round trips of phase 3.
                nextcall = [0] * BANKS
                gtiles = [dict() for _ in range(BANKS)]
                stg = {"h": None, "o": None}

                def pass_a(w):
                    for b in range(BANKS):
                        while nextcall[b] * PAGE < min((w + 1) * K, CPB):
                            j = nextcall[b]
                            gt = gpool.tile([P, PAGE, ROW], F16, tag=f"g{b}")
                            col0 = (b * NCALLB + j) * S
                            nc.gpsimd.dma_gather(
                                gt[:], table[L][BANK_BASES[b]:BANK_BASES[b] + BANK_ROWS, :],
                                idx_t[:, col0:col0 + S], NIDX, NIDX, ROW,
                                queue_num=b)
                            gtiles[b][j] = gt
                            if j - 2 in gtiles[b]:
                                del gtiles[b][j - 2]
                            nextcall[b] += 1
                    eps = epsp.tile([P, CW * H], F32, tag="ep")
                    ohs = []
                    for cw in range(CW):
                        cid = (w * BANKS + (cw // K)) * K + (cw % K)
                        oh = ohpool.tile([P, P], F16, tag="oh")
                        eng = nc.gpsimd if cw >= CW - K else nc.vector
                        eng.tensor_scalar(
                            oh[:], iota_row[:], rel_t[:, cid:cid + 1], None,
                            op0=ALU.is_equal)
                        ohs.append(oh)
                    ohTs = []
                    for b in range(BANKS):
                        trp = trps.tile([P, K * P], F16, tag="tr")
                        for k in range(K):
                            nc.tensor.transpose(trp[:, k * P:(k + 1) * P],
                                                ohs[b * K + k][:], ident[:])
                        ohT = cmppool.tile([P, K * P], F16, tag="ohT")
                        nc.scalar.activation(ohT[:], trp[:], AF.Copy)
                        ohTs.append(ohT)
                    # paired open/close per chunk: PSUM tracks only one open
                    # accumulation group per bank, so each chunk's two matmuls
                    # (ohT.T@adres then I.T@rows_as) must be adjacent.
                    for cw in range(CW):
                        b, k = cw // K, cw % K
                        cglob = w * K + k
                        gt = gtiles[b][cglob // PAGE]
                        nc.tensor.matmul(eps[:, cw * H:(cw + 1) * H],
                                         lhsT=ohTs[b][:, k * P:(k + 1) * P],
                                         rhs=adres[L][:, w * H:(w + 1) * H],
                                         start=True, stop=False)
                        nc.tensor.matmul(eps[:, cw * H:(cw + 1) * H],
                                         lhsT=ident[:],
                                         rhs=gt[:, cglob % PAGE, 132:136],
                                         start=False, stop=True)
                    return eps, ohs

                def pass_b(w, eps, ohs):
                    # exp(lrelu(e)) = max(exp(e), exp(0.2 e)) since exp is
                    # monotonic: two scalar-engine exps + one f16 vector max
                    exa = p3pool.tile([P, CW * H], F16, tag="exa")
                    nc.scalar.activation(exa[:], eps[:], AF.Exp)
                    exb = p3pool.tile([P, CW * H], F16, tag="exb")
                    nc.scalar.activation(exb[:], eps[:], AF.Exp, scale=NEG_SLOPE)
                    ex = p3pool.tile([P, CW * H], F16, tag="ex")
                    nc.vector.tensor_tensor(ex[:], exa[:], exb[:], op=ALU.max)
                    pw = wps.tile([P, RW], F32, tag="pw")
                    # fused rhs over runs of chunks sharing one gather tile
                    for b in range(BANKS):
                        k = 0
                        while k < K:
                            cglob = w * K + k
                            j = cglob // PAGE
                            pg = cglob % PAGE
                            kn = min(K - k, PAGE - pg)
                            cw0 = b * K + k
                            rows3 = gtiles[b][j][:, pg:pg + kn, 0:132]
                            rhs = cmppool.tile([P, K, 132], F16, tag="rhs")
                            nc.vector.tensor_tensor(
                                rhs[:, 0:kn, :].rearrange("p k (h c) -> p k h c", h=H),
                                rows3.rearrange("p k (h c) -> p k h c", h=H),
                                ex[:, cw0 * H:(cw0 + kn) * H].rearrange(
                                    "p (k h) -> p k h", h=H)[:, :, :, None]
                                    .to_broadcast([P, kn, H, 33]),
                                op=ALU.mult)
                            for kk in range(kn):
                                cw = cw0 + kk
                                nc.tensor.matmul(pw[:, 0:132], lhsT=ohs[cw][:],
                                                 rhs=rhs[:, kk, :],
                                                 start=(cw == 0), stop=(cw == CW - 1))
                            k += kn
                    return pw

                def phase3(w, pw):
                    sp = p3pool.tile([P, H], F32, tag="s")
                    nc.scalar.activation(
                        sp[:, :, None],
                        pw[:, 0:132].rearrange("p (h c) -> p h c", h=H)[:, :, 0:1],
                        AF.Copy, bias=EPS)
                    r = p3pool.tile([P, H], F32, tag="r")
                    nc.vector.reciprocal(r[:], sp[:])
                    hw = p3pool.tile([P, F], F32, tag="hw")
                    nc.vector.tensor_tensor(
                        hw[:].rearrange("p (h c) -> p h c", h=H),
                        pw[:, 0:132].rearrange("p (h c) -> p h c", h=H)[:, :, 1:33],
                        r[:, :, None].to_broadcast([P, H, C]),
                        op=ALU.mult)
                    if L == 0:
                        if w % 8 == 0:
                            hs_new = stgpool.tile([P, 1024], F16, tag="hs")
                            stg["h"] = hs_new
                        hstage = stg["h"]
                        nc.vector.tensor_tensor(hw[:], hw[:], b1_t[:], op=ALU.add)
                        # elu
                        mn = p3pool.tile([P, F], F32, tag="mn")
                        nc.vector.tensor_scalar(mn[:], hw[:], 0.0, None, op0=ALU.min)
                        mx = p3pool.tile([P, F], F32, tag="mx")
                        nc.vector.tensor_scalar(mx[:], hw[:], 0.0, None, op0=ALU.max)
                        ek = p3pool.tile([P, F], F32, tag="ek")
                        nc.scalar.activation(ek[:], mn[:], AF.Exp)
                        he = p3pool.tile([P, F], F16, tag="he")
                        nc.vector.scalar_tensor_tensor(
                            he[:], ek[:], -1.0, mx[:], op0=ALU.add, op1=ALU.add)
                        trp = trps.tile([P, K * P], F16, tag="tr")
                        nc.tensor.transpose(trp[:, 0:P], he[:], ident[:])
                        hcol = (w % 8) * P
                        nc.scalar.activation(hstage[:, hcol:hcol + P], trp[:, 0:P],
                                             AF.Copy)
                        adp = epsp.tile([P, CW * H], F32, tag="ep")
                        nc.tensor.matmul(adp[:, 0:H],
                                         lhsT=hstage[:, hcol:hcol + P], rhs=wad[1][:],
                                         start=True, stop=True)
                        nc.vector.tensor_copy(adres1[:, w * H:(w + 1) * H], adp[:, 0:H])
                        if w % 8 == 7 or w == WPC - 1:
                            g0 = (w // 8) * 1024
                            gw = min(1024, MPAD - g0)
                            nc.sync.dma_start(out=h_shard[:, g0:g0 + gw],
                                              in_=hstage[:, 0:gw])
                    else:
                        if w % 8 == 0:
                            os_new = stgpool.tile([P, NT8, C], F32, tag="os")
                            stg["o"] = os_new
                        ostage = stg["o"]
                        red = p3pool.tile([P, C], F32, tag="red")
                        nc.vector.tensor_reduce(
                            red[:], hw[:].rearrange("p (h c) -> p c h", h=H),
                            axis=mybir.AxisListType.X, op=ALU.add)
                        nc.vector.scalar_tensor_tensor(
                            ostage[:, w % 8, :], red[:], 1.0 / H, b2_t[:],
                            op0=ALU.mult, op1=ALU.add)
                        if w % 8 == 7 or w == WPC - 1:
                            g0 = (w // 8) * 1024
                            kw = (w % 8) + 1
                            nc.sync.dma_start(
                                out=out2[g0:g0 + kw * P, :].rearrange(
                                    "(k p) c -> p k c", p=P),
                                in_=ostage[:, 0:kw, :])

                prev = None
                for w in range(WPC):
                    eps, ohs = pass_a(w)
                    if prev is not None:
                        phase3(w - 1, prev)
                    prev = pass_b(w, eps, ohs)
                phase3(WPC - 1, prev)

                if L == 0:
                    nc.gpsimd.collective_compute(
                        "AllGather", ALU.bypass,
                        replica_groups=[list(range(CORES))],
                        ins=[h_shard.ap()],
                        outs=[h_full.ap()],
                    )
    nc.compile()
    _NC_CACHE[ck] = nc
    return nc


def make_inmaps(inputs, perm, consts):
    x = np.asarray(inputs["x"], np.float32)
    wcat1, wad1 = _pack_mats(np.asarray(inputs["W1"], np.float32),
                             np.asarray(inputs["att_src1"]), np.asarray(inputs["att_dst1"]))
    wcat2, wad2 = _pack_mats(np.asarray(inputs["W2"], np.float32),
                             np.asarray(inputs["att_src2"]), np.asarray(inputs["att_dst2"]))
    b1 = np.asarray(inputs["b1"], np.float32)
    b2 = np.asarray(inputs["b2"], np.float32)
    brow_np = np.zeros((1, RW), np.float16)
    for h in range(H):
        brow_np[0, h * 33] = 1.0
    xT_np = np.zeros((P, NPAD), np.float16)
    xT_np[:, :N] = np.ascontiguousarray(x.T).astype(np.float16)
    common = {
        "xT": xT_np,
        "wcat1": wcat1.astype(np.float16), "wcat2": wcat2.astype(np.float16),
        "wad1": wad1.astype(np.float16), "wad2": wad2.astype(np.float16),
        "brow": brow_np,
        "b1t": np.tile(b1[None, :], (P, 1)).astype(np.float32),
        "b2t": np.tile(b2[None, :], (P, 1)).astype(np.float32),
    }
    maps = []
    for m in range(CORES):
        im = dict(common)
        xs = np.zeros((P, XSPAD), np.float16)
        xs[:, :M] = xT_np[:, m * M:(m + 1) * M]
        im["xTs"] = xs
        im["idxw"] = perm[m]["idxw"]
        im["dstrel"] = perm[m]["dstrel"]
        maps.append(im)
    return maps


def run_on_hw(inputs, perm, consts):
    nc = build(consts)
    maps = make_inmaps(inputs, perm, consts)
    res = run_bass_kernel_spmd(nc, maps, core_ids=list(range(CORES)))
    return np.concatenate([res.results[m]["out2"][:M] for m in range(CORES)], axis=0)


def kernel(**inputs):
    perm, consts = _host_prep(np.asarray(inputs["edge_index"]))
    out = run_on_hw(inputs, perm, consts)
    if not np.isfinite(out).all():
        # transient first-dispatch flakiness: retry once
        out = run_on_hw(inputs, perm, consts)
    return out
